# revision 1
# baseline (speedup 1.0000x reference)
"""Trainium2 Bass kernel for a 2x-MHA + FFN transformer block (fused pipeline).

Contract: kernel(**inputs) takes FULL unsharded inputs (numpy) and returns the
FULL output [1024, 32, 1024] float32.

Verified on HW: 1.940 ms (baseline 3.067 ms), absmax-rel err 7.8e-3.

Strategy:
  - Pure data-parallel over batch B=1024 across 8 NeuronCores (128 batches =
    4096 tokens per core). No collectives.
  - Transposed activations on device: xT [E, tokens] so every dense matmul is
    out = lhsT.T @ rhs with K (contraction) on partitions.
  - Three passes per core, each with its weights SBUF-RESIDENT, streaming the
    4096 tokens in 8 chunks of 512:
      pass 1: layer-1  qk -> attention -> proj(+residual)
      pass 2: layer-2  same
      pass 3: FFN      relu(x@W1+b1)@W2 + b2 (+residual), hT stays in SBUF
  - qk projections run in fp8 e4m3 DoubleRow (2 K-rows/cycle; weights
    pre-scaled x64 on host, descaled in the eviction).  fp8 elsewhere fails
    the 2e-2 gate (measured: ffn 2.7e-2, +v 2.0e-2); qk costs only +5e-4
    because softmax damps logit noise.
  - Residual spine is bf16 (error budget allows; halves DMA + SBUF).
  - Attention is batched 4 heads per step: scores for 4 SAME-PARITY heads
    (mixed 0/64 base-partitions into one PSUM bank crash the HW stack) land
    in ONE [128,512] PSUM bank; exp/mask/normalize run as [128,512]-wide
    ACT/DVE ops (amortizes the ~150-250ns fixed per-instruction engine cost).
    One STREAM_TRANSPOSE per 4 heads: wei is 32-block-diagonal (4 batches of
    T=32), so the DVE's per-32x32-block transpose IS the full transpose.
  - Software pipelining: iteration c interleaves, at PE-unit granularity, the
    dense front of chunk c (qk+v), attention of chunk c-1 (out-matmuls trail
    their scores by two 4-head batches so the softmax DVE chain never
    head-of-line blocks the PE queue), and the projection of chunk c-2.
  - v-bias folded into the proj bias on host (softmax rows sum to 1).
  - attn^T staged through per-parity DRAM tensors (the only way to move head
    outputs from partitions 0:64 into the 128-partition proj K-layout).
  - Pass-boundary overlap: pools shared across both layer passes (slots free
    as soon as the last user finishes, not at pass end), per-chunk DRAM
    tensors for the inter-pass activations (no whole-tensor dependencies),
    and a quarter of W_ff1 preloaded into SBUF slack during pass 2.
"""
import sys

if "/opt/trn_rl_repo" not in sys.path:
    sys.path.insert(0, "/opt/trn_rl_repo")

from contextlib import ExitStack

import numpy as np

import concourse.bacc as bacc
import concourse.mybir as mybir
import concourse.tile as tile

F32 = mybir.dt.float32
BF16 = mybir.dt.bfloat16
F8E4 = mybir.dt.float8e4
P = 128

QK_FP8 = True      # qk matmuls in e4m3 DoubleRow (2 K-rows/cycle)
W8_SCALE = 64.0    # weight pre-scale into e4m3 sweet spot; descaled at evict

E = 1024
H = 16
D = 64
T = 32
HD = H * D  # 1024
FF = 4 * E  # 4096
N_CORES = 8
B_FULL = 1024
C = 512  # token chunk (columns per dense matmul)

AF = mybir.ActivationFunctionType
ALU = mybir.AluOpType

# bisection flags (all-True = fastest path)
FLAGS = {
    "big_transpose": True,   # one [128,512] STREAM_TRANSPOSE vs 4x[128,128]
    "scores_onebank": True,  # 4 scores MMs into one PSUM bank vs 4 banks
    "do_attn": True,         # emit attention (else memset ostrip)
    "batched_evict": True,   # one [64,512] ACT evict vs 4x[64,128]
}

# Head processing order: 4-head batches of UNIFORM parity, so the 4 scores
# matmuls sharing one PSUM bank all use the same tile_position (mixed 0/64
# row-positions into one bank crash the HW stack).
HEAD_BATCHES = [(0, 2, 4, 6), (8, 10, 12, 14), (1, 3, 5, 7), (9, 11, 13, 15)]


def _rd(ap):
    """DRAM [R, t] viewed as [128, R//128, t] (row r = k*128 + p)."""
    return ap.rearrange("(k p) t -> p k t", p=P)


def _chunk(ap_or_list, c):
    """Chunk c of an activation: per-chunk DRAM tensor list or one tensor."""
    if isinstance(ap_or_list, list):
        return _rd(ap_or_list[c])
    return _rd(ap_or_list)[:, :, slice(c * C, (c + 1) * C)]


def _layer_pass(tc, psums, pools, mask_sb, xin_d, Wqk_d, Wv_d, Wp_d,
                bqk_d, bp_d, stage_d, xout_d, tok):
    """One attention layer: xout = xin + attn(xin) @ Wp + bp' (all bf16 I/O).

    Software-pipelined over chunks: iteration c emits, PE-interleaved at unit
    granularity, the dense front of chunk c (qk+v), the attention of chunk
    c-1, and the projection of chunk c-2.  Attention out-matmuls trail their
    scores by one 4-head batch so the softmax DVE chain never head-of-line
    blocks the PE queue.
    """
    nc = tc.nc
    mm_ps, s_ps_pool, o_ps_pool = psums
    n_chunks = tok // C
    n_grp = C // P  # groups of 4 batches per chunk
    scale = float(D) ** -0.5

    if True:
        wp = pools["wts"]
        Wqk_sb = wp.tile([P, E // P, 2 * HD], F8E4 if QK_FP8 else BF16,
                         name="Wqk_sb")
        bqk_sb = wp.tile([P, 16], F32, name="bqk_sb")
        nc.sync.dma_start(bqk_sb[:], bqk_d[:])
        bp_sb = wp.tile([P, 8], F32, name="bp_sb")
        nc.sync.dma_start(bp_sb[:], bp_d[:])
        # Wv/Wp loads are deferred to just after the first x-chunk DMA so the
        # first qk matmuls aren't stuck behind them in the DMA queue.
        Wv_sb = wp.tile([P, HD // P, HD], BF16, name="Wv_sb")
        Wp_sb = wp.tile([P, HD // P, E], BF16, name="Wp_sb")

        x_pool = pools["x"]
        x8_pool = pools["x8"]
        qkT_pool = pools["qkT"]
        v_pool = pools["v"]
        w_pool = pools["w"]
        wT_pool = pools["wT"]
        st_pool = pools["st"]
        os_pool = pools["os"]
        aT_pool = pools["aT"]
        xo_pool = pools["xo"]

        x_tiles, x8_tiles, qkT_tiles, v_tiles, aT_tiles = {}, {}, {}, {}, {}

        def qk_unit(c, m):
            ps = mm_ps.tile([P, C], F32, name="mm")
            if QK_FP8:
                x8 = x8_tiles[c]
                for k in range(4):
                    nc.tensor.matmul(
                        ps[:],
                        lhsT=Wqk_sb[:, 2 * k:2 * k + 2, m * P:(m + 1) * P],
                        rhs=x8[:, 2 * k:2 * k + 2, :],
                        perf_mode=mybir.MatmulPerfMode.DoubleRow,
                        start=(k == 0), stop=(k == 3),
                    )
                nc.vector.tensor_scalar(
                    qkT_tiles[c][:, m, :], ps, 1.0 / W8_SCALE,
                    bqk_sb[:, m:m + 1], ALU.mult, ALU.add)
            else:
                for k in range(8):
                    nc.tensor.matmul(
                        ps[:], lhsT=Wqk_sb[:, k, m * P:(m + 1) * P],
                        rhs=x_tiles[c][:, k, :], start=(k == 0), stop=(k == 7),
                    )
                nc.vector.tensor_scalar_add(
                    qkT_tiles[c][:, m, :], ps, bqk_sb[:, m:m + 1])

        def v_unit(c, mt, n2):
            ps = mm_ps.tile([P, C], F32, name="mm")
            for k in range(8):
                nc.tensor.matmul(
                    ps[:], lhsT=x_tiles[c][:, k, mt * P:(mt + 1) * P],
                    rhs=Wv_sb[:, k, n2 * C:(n2 + 1) * C],
                    start=(k == 0), stop=(k == 7),
                )
            nc.vector.tensor_copy(
                out=v_tiles[c][:, mt, n2 * C:(n2 + 1) * C], in_=ps)

        def proj_unit(c, m):
            ps = mm_ps.tile([P, C], F32, name="mm")
            for k in range(8):
                nc.tensor.matmul(
                    ps[:], lhsT=Wp_sb[:, k, m * P:(m + 1) * P],
                    rhs=aT_tiles[c][:, k, :], start=(k == 0), stop=(k == 7),
                )
            xo = xo_tiles[c]
            nc.vector.tensor_tensor(
                out=xo[:, m, :], in0=ps, in1=x_tiles[c][:, m, :], op=ALU.add)
            nc.vector.tensor_scalar_add(
                xo[:, m, :], xo[:, m, :], bp_sb[:, m:m + 1])

        xo_tiles = {}

        def scores_front(c, g, hb):
            """4 scores MMs + softmax chain; returns weiT tile."""
            qkT = qkT_tiles[c]
            gc = slice(g * P, (g + 1) * P)
            heads = HEAD_BATCHES[hb]
            s_ps = s_ps_pool.tile([P, 4 * P], F32, name="s_ps")
            for j, h in enumerate(heads):
                r0 = D * (h % 2)
                nc.tensor.matmul(
                    s_ps[:, j * P:(j + 1) * P],
                    lhsT=qkT[r0:r0 + D, h // 2, gc],
                    rhs=qkT[r0:r0 + D, 8 + h // 2, gc],
                    start=True, stop=True,
                )
            e_sb = w_pool.tile([P, 4 * P], BF16, name="e_sb")
            nc.scalar.activation(e_sb, s_ps, AF.Exp, scale=scale)
            wei = w_pool.tile([P, 4 * P], BF16, name="wei")
            nc.vector.tensor_tensor(out=wei, in0=e_sb, in1=mask_sb, op=ALU.mult)
            sums = st_pool.tile([P, 4], F32, name="sums")
            nc.vector.tensor_reduce(
                out=sums, in_=wei.rearrange("p (j t) -> p j t", j=4),
                op=ALU.add, axis=mybir.AxisListType.X,
            )
            rcp = st_pool.tile([P, 4], F32, name="rcp")
            nc.vector.reciprocal(rcp, sums)
            nc.vector.tensor_tensor(
                out=wei.rearrange("p (j t) -> p j t", j=4),
                in0=wei.rearrange("p (j t) -> p j t", j=4),
                in1=rcp.unsqueeze(-1).broadcast_to([P, 4, P]),
                op=ALU.mult,
            )
            weiT = wT_pool.tile([P, 4 * P], BF16, name="weiT")
            nc.vector.transpose(weiT, wei)  # 32-blockwise == full T here
            return weiT

        def out_back(c, g, hb, weiT):
            """4 out MMs + strip eviction (+ stage DMA after last hb of g)."""
            heads = HEAD_BATCHES[hb]
            if hb == 0:
                ostrips[(c, g)] = os_pool.tile([D, H, P], BF16, name="ostrip")
            ostrip = ostrips[(c, g)]
            o_ps = o_ps_pool.tile([P, 4 * P], F32, name="o_ps")
            for j, h in enumerate(heads):
                nc.tensor.matmul(
                    o_ps[0:D, j * P:(j + 1) * P],
                    lhsT=v_tiles[c][:, g, h * D:(h + 1) * D],
                    rhs=weiT[:, j * P:(j + 1) * P],
                    start=True, stop=True,
                )
            nc.scalar.activation(
                ostrip[:, heads[0]:heads[-1] + 1:2, :],
                o_ps[0:D, :].rearrange("p (j t) -> p j t", j=4),
                AF.Copy,
            )
            if hb == 3:
                gc = slice(g * P, (g + 1) * P)
                nc.sync.dma_start(
                    stage_d[c % 2].rearrange("(h d) t -> d h t", d=D)[:, :, gc],
                    ostrip[:],
                )
                del ostrips[(c, g)]
                # read this group's columns straight back (partition shuffle);
                # per-group readback maximizes DMA lead time for proj
                if c not in aT_tiles:
                    aT_tiles[c] = aT_pool.tile([P, HD // P, C], BF16, name="aT")
                nc.sync.dma_start(
                    aT_tiles[c][:, :, gc], _rd(stage_d[c % 2])[:, :, gc])

        ostrips = {}

        def load_x(c):
            if c in x_tiles or c >= n_chunks:
                return
            x_sb = x_pool.tile([P, E // P, C], BF16, name="x_sb")
            nc.sync.dma_start(x_sb[:], _chunk(xin_d, c))
            x_tiles[c] = x_sb
            if c == 0:
                nc.sync.dma_start(Wqk_sb[:, :, 0:HD], _rd(Wqk_d)[:, :, 0:HD])
                nc.sync.dma_start(Wqk_sb[:, :, HD:2 * HD], _rd(Wqk_d)[:, :, HD:2 * HD])
                nc.sync.dma_start(Wv_sb[:], _rd(Wv_d))
                nc.sync.dma_start(Wp_sb[:], _rd(Wp_d))
            if QK_FP8:
                x8 = x8_pool.tile([P, E // P, C], F8E4, name="x8")
                for kq in range(4):
                    nc.vector.tensor_copy(
                        out=x8[:, 2 * kq:2 * kq + 2, :],
                        in_=x_sb[:, 2 * kq:2 * kq + 2, :])
                x8_tiles[c] = x8

        for c in range(n_chunks + 2):
            dense = []
            if c < n_chunks:
                load_x(c)
                load_x(c + 1)  # prefetch: next chunk's DMA a full iter early
                qkT_tiles[c] = qkT_pool.tile([P, 16, C], BF16, name="qkT")
                v_tiles[c] = v_pool.tile([P, n_grp, HD], BF16, name="V")
                dense += [(qk_unit, (c, m)) for m in range(16)]
                dense += [(v_unit, (c, mt, n2))
                          for mt in range(n_grp) for n2 in range(HD // C)]
            if c >= 2:
                xo_tiles[c - 2] = xo_pool.tile([P, E // P, C], BF16, name="xo")
                dense += [(proj_unit, (c - 2, m)) for m in range(8)]

            if 1 <= c <= n_chunks:
                ca = c - 1
                pend = []
                for g in range(n_grp):
                    for hb in range(4):
                        weiT = scores_front(ca, g, hb)
                        for _ in range(2):
                            if dense:
                                fn, args = dense.pop(0)
                                fn(*args)
                        if len(pend) >= 2:
                            out_back(*pend.pop(0))
                        pend.append((ca, g, hb, weiT))
                for fn, args in dense:
                    fn(*args)
                for pd in pend:
                    out_back(*pd)
                qkT_tiles.pop(ca)
                v_tiles.pop(ca)
                x8_tiles.pop(ca, None)
            else:
                for fn, args in dense:
                    fn(*args)

            if c >= 2:
                cp = c - 2
                nc.sync.dma_start(_chunk(xout_d, cp), xo_tiles.pop(cp)[:])
                x_tiles.pop(cp)
                aT_tiles.pop(cp)


def _ffn_pass(tc, psums, pools, preload, xin_d, Wff1_d, Wff2_d, bff1_d, bff2_d, out_d, tok):
    """out = xin + relu(xin @ W1 + b1) @ W2 + b2   (out is fp32)."""
    nc = tc.nc
    mm_ps, _, _ = psums
    n_chunks = tok // C

    with ExitStack() as ctx:
        W1a_sb, b1_sb, b2_sb = preload
        wp = ctx.enter_context(tc.tile_pool(name="fwts", bufs=1))
        x_pool = pools["x"]
        h_pool = ctx.enter_context(tc.tile_pool(name="fh_pool", bufs=1))
        o_pool = ctx.enter_context(tc.tile_pool(name="fo_pool", bufs=3))

        # first x chunk before the remaining big weight loads
        x0_sb = x_pool.tile([P, E // P, C], BF16, name="x_sb")
        nc.sync.dma_start(x0_sb[:], _chunk(xin_d, 0))
        W1b_sb = wp.tile([P, E // P, 3 * FF // 4], BF16, name="W1b_sb")
        nc.sync.dma_start(W1b_sb[:], _rd(Wff1_d)[:, :, FF // 4:FF])
        # W2 is not needed until the first ffn2 (~55us in); load it behind W1
        W2_sb = wp.tile([P, FF // P, E], BF16, name="W2_sb")
        nc.sync.dma_start(W2_sb[:], _rd(Wff2_d))

        fx_tiles = {0: x0_sb}

        def fload_x(c):
            if c in fx_tiles or c >= n_chunks:
                return
            t = x_pool.tile([P, E // P, C], BF16, name="x_sb")
            nc.sync.dma_start(t[:], _chunk(xin_d, c))
            fx_tiles[c] = t

        for c in range(n_chunks):
            cols = slice(c * C, (c + 1) * C)
            fload_x(c)
            fload_x(c + 1)
            x_sb = fx_tiles[c]

            hT = h_pool.tile([P, FF // P, C], BF16, name="hT")
            for m in range(FF // P):
                W1h, mh = (W1a_sb, m) if m < 8 else (W1b_sb, m - 8)
                ps = mm_ps.tile([P, C], F32, name="mm")
                for k in range(8):
                    nc.tensor.matmul(
                        ps[:], lhsT=W1h[:, k, mh * P:(mh + 1) * P],
                        rhs=x_sb[:, k, :], start=(k == 0), stop=(k == 7),
                    )
                nc.scalar.activation(
                    hT[:, m, :], ps, AF.Relu, bias=b1_sb[:, m:m + 1])

            for m in range(E // P):
                ps = mm_ps.tile([P, C], F32, name="mm")
                for k in range(FF // P):
                    nc.tensor.matmul(
                        ps[:], lhsT=W2_sb[:, k, m * P:(m + 1) * P],
                        rhs=hT[:, k, :], start=(k == 0), stop=(k == FF // P - 1),
                    )
                ot = o_pool.tile([P, C], F32, name="ot")
                nc.vector.tensor_scalar_add(ot, ps, b2_sb[:, m:m + 1])
                nc.vector.tensor_tensor(
                    out=ot, in0=ot, in1=x_sb[:, m, :], op=ALU.add)
                nc.sync.dma_start(_rd(out_d)[:, m, cols], ot[:])


def build_kernel(b_shard):
    """Build the per-core Bass module for a batch shard of b_shard blocks."""
    tok = b_shard * T
    nc = bacc.Bacc(None, target_bir_lowering=False)
    with tile.TileContext(nc) as tc:
        with ExitStack() as ctx:
            dram = ctx.enter_context(tc.tile_pool(name="dram", bufs=1, space="DRAM"))

            def din(name, shape, dt=BF16):
                return dram.tile(shape, dt, kind="ExternalInput", uniquify=False, name=name)

            qk_dt = F8E4 if QK_FP8 else BF16
            xTb = din("xTb", [E, tok])
            Wqk1 = din("Wqk1", [E, 2 * HD], qk_dt); Wv1 = din("Wv1", [E, HD]); Wp1 = din("Wp1", [HD, E])
            Wqk2 = din("Wqk2", [E, 2 * HD], qk_dt); Wv2 = din("Wv2", [E, HD]); Wp2 = din("Wp2", [HD, E])
            Wff1 = din("Wff1", [E, FF]); Wff2 = din("Wff2", [FF, E])
            bqk1 = din("bqk1", [P, 16], F32); bp1 = din("bp1", [P, 8], F32)
            bqk2 = din("bqk2", [P, 16], F32); bp2 = din("bp2", [P, 8], F32)
            bff1 = din("bff1", [P, 32], F32); bff2 = din("bff2", [P, 8], F32)
            maskc = din("maskc", [P, 4 * P])

            outT = dram.tile([E, tok], F32, kind="ExternalOutput", uniquify=False, name="outT")

            n_ch = tok // C
            xT2b = [dram.tile([E, C], BF16, kind="Internal", uniquify=False, name=f"xT2b_{i}") for i in range(n_ch)]
            xT3b = [dram.tile([E, C], BF16, kind="Internal", uniquify=False, name=f"xT3b_{i}") for i in range(n_ch)]
            stg1 = [dram.tile([HD, C], BF16, kind="Internal", uniquify=False, name=f"stg1_{i}") for i in range(2)]
            stg2 = [dram.tile([HD, C], BF16, kind="Internal", uniquify=False, name=f"stg2_{i}") for i in range(2)]

            sb = 2 if FLAGS["scores_onebank"] else 1
            mm_ps = ctx.enter_context(tc.tile_pool(name="mm_ps", bufs=4, space="PSUM"))
            s_ps = ctx.enter_context(tc.tile_pool(name="s_ps", bufs=sb, space="PSUM"))
            o_ps = ctx.enter_context(tc.tile_pool(name="o_ps", bufs=sb, space="PSUM"))
            psums = (mm_ps, s_ps, o_ps)

            pools = {
                "x": ctx.enter_context(tc.tile_pool(name="x_pool", bufs=4)),
            }
            const_p = ctx.enter_context(tc.tile_pool(name="const", bufs=1))
            mask_sb = const_p.tile([P, 4 * P], BF16, name="mask_sb")
            nc.sync.dma_start(mask_sb[:], maskc[:])
            fpre = ctx.enter_context(tc.tile_pool(name="fwts_pre", bufs=1))

            with ExitStack() as lctx:
                for nm, bufs in (("wts", 1), ("x8", 2), ("qkT", 2), ("v", 2),
                                 ("w", 3), ("wT", 3), ("st", 6), ("os", 3),
                                 ("aT", 2), ("xo", 2)):
                    pools[nm] = lctx.enter_context(
                        tc.tile_pool(name=nm + "_pool", bufs=bufs))
                _layer_pass(tc, psums, pools, mask_sb, xTb, Wqk1, Wv1, Wp1, bqk1, bp1, stg1, xT2b, tok)
                # preload part of the FFN weights during layer pass 2 (fits
                # in SBUF slack; sits early in the DMA queue)
                W1a_sb = fpre.tile([P, E // P, FF // 4], BF16, name="W1a_sb")
                nc.sync.dma_start(W1a_sb[:], _rd(Wff1)[:, :, 0:FF // 4])
                b1_sb = fpre.tile([P, FF // P], F32, name="b1_sb")
                nc.sync.dma_start(b1_sb[:], bff1[:])
                b2_sb = fpre.tile([P, E // P], F32, name="b2_sb")
                nc.sync.dma_start(b2_sb[:], bff2[:])
                _layer_pass(tc, psums, pools, mask_sb, xT2b, Wqk2, Wv2, Wp2, bqk2, bp2, stg2, xT3b, tok)
            _ffn_pass(tc, psums, pools, (W1a_sb, b1_sb, b2_sb), xT3b, Wff1, Wff2, bff1, bff2, outT, tok)

    nc.compile()
    return nc


# --------------------------------------------------------------------------
# Host-side wrapper
# --------------------------------------------------------------------------

import ml_dtypes

BF16_NP = ml_dtypes.bfloat16


def _w_heads(W):
    """[H, E, D] -> [E, H*D] contiguous bf16 (col = 64h + d)."""
    return np.ascontiguousarray(
        np.transpose(np.asarray(W), (1, 0, 2)).reshape(E, HD).astype(BF16_NP)
    )


def _b_tile(b, n_po):
    """[Dim] -> [128, n_po] bias tile (row r = po*128 + pi)."""
    b = np.asarray(b, dtype=np.float32).reshape(n_po, P)
    return np.ascontiguousarray(b.T)


def _causal_mask_tile():
    """0/1 bf16 mask [128, 512]: 4-head tile of block-diag causal [128,128]."""
    m = np.zeros((P, P), dtype=np.float32)
    for p in range(P):
        blk, t = p // T, p % T
        m[p, blk * T: blk * T + t + 1] = 1.0
    return np.ascontiguousarray(np.tile(m, (1, 4)).astype(BF16_NP))


def make_in_maps(inputs, b_shard=B_FULL // N_CORES, n_cores=N_CORES):
    x = np.asarray(inputs["x"], dtype=np.float32)
    shared = {
        "maskc": _causal_mask_tile(),
        "Wff1": np.asarray(inputs["W_ff1"], np.float32).astype(BF16_NP),
        "Wff2": np.asarray(inputs["W_ff2"], np.float32).astype(BF16_NP),
        "bff1": _b_tile(inputs["b_ff1"], 32),
        "bff2": _b_tile(inputs["b_ff2"], 8),
    }
    F8_NP = ml_dtypes.float8_e4m3
    for li in ("1", "2"):
        Wq = _w_heads(inputs["Wq" + li])
        Wk = _w_heads(inputs["Wk" + li])
        wqk = np.ascontiguousarray(np.concatenate([Wq, Wk], axis=1))
        if QK_FP8:
            wqk = np.clip(wqk.astype(np.float32) * W8_SCALE, -240, 240).astype(F8_NP)
        shared["Wqk" + li] = wqk
        shared["Wv" + li] = _w_heads(inputs["Wv" + li])
        Wp = np.asarray(inputs["Wp" + li], np.float32)
        shared["Wp" + li] = np.ascontiguousarray(Wp.astype(BF16_NP))
        bq = np.asarray(inputs["bq" + li], np.float32).reshape(HD)
        bk = np.asarray(inputs["bk" + li], np.float32).reshape(HD)
        shared["bqk" + li] = _b_tile(np.concatenate([bq, bk]), 16)
        # fold v-bias through the projection:  bp' = bp + bv @ Wp
        bv = np.asarray(inputs["bv" + li], np.float32).reshape(HD)
        bp = np.asarray(inputs["bp" + li], np.float32) + bv @ Wp
        shared["bp" + li] = _b_tile(bp, 8)

    in_maps = []
    for c in range(n_cores):
        xs = x[c * b_shard:(c + 1) * b_shard].reshape(b_shard * T, E)
        m = dict(shared)
        m["xTb"] = np.ascontiguousarray(xs.T.astype(BF16_NP))
        in_maps.append(m)
    return in_maps


_NC_CACHE = {}


def kernel(**inputs) -> np.ndarray:
    from concourse.bass_utils import run_bass_kernel_spmd

    b_shard = B_FULL // N_CORES
    if b_shard not in _NC_CACHE:
        _NC_CACHE[b_shard] = build_kernel(b_shard)
    nc = _NC_CACHE[b_shard]

    in_maps = make_in_maps(inputs)
    res = run_bass_kernel_spmd(nc, in_maps, core_ids=list(range(N_CORES)))

    out = np.empty((B_FULL, T, E), dtype=np.float32)
    for c in range(N_CORES):
        outT = res.results[c]["outT"]  # [E, tok]
        out[c * b_shard:(c + 1) * b_shard] = outT.T.reshape(b_shard, T, E)
    return out



# revision 21
# speedup vs baseline: 1.0746x; 1.0746x over previous
"""Trainium2 Bass kernel for a 2x-MHA + FFN transformer block (fused pipeline).

Contract: kernel(**inputs) takes FULL unsharded inputs (numpy) and returns the
FULL output [1024, 32, 1024] float32.

Verified on HW: 1.940 ms (baseline 3.067 ms), absmax-rel err 7.8e-3.
v2: FFN split-K fp8 (rows < FFN*_K8 in e4m3 DoubleRow, both halves' weights
x64 so one PSUM scale; evict descales), layer-2/FFN weight prefetch,
alternating-parity head batches, fused proj eviction, drain rebalance.

Strategy:
  - Pure data-parallel over batch B=1024 across 8 NeuronCores (128 batches =
    4096 tokens per core). No collectives.
  - Transposed activations on device: xT [E, tokens] so every dense matmul is
    out = lhsT.T @ rhs with K (contraction) on partitions.
  - Three passes per core, each with its weights SBUF-RESIDENT, streaming the
    4096 tokens in 8 chunks of 512:
      pass 1: layer-1  qk -> attention -> proj(+residual)
      pass 2: layer-2  same
      pass 3: FFN      relu(x@W1+b1)@W2 + b2 (+residual), hT stays in SBUF
  - qk projections run in fp8 e4m3 DoubleRow (2 K-rows/cycle; weights
    pre-scaled x64 on host, descaled in the eviction).  fp8 elsewhere fails
    the 2e-2 gate (measured: ffn 2.7e-2, +v 2.0e-2); qk costs only +5e-4
    because softmax damps logit noise.
  - Residual spine is bf16 (error budget allows; halves DMA + SBUF).
  - Attention is batched 4 heads per step: scores for 4 SAME-PARITY heads
    (mixed 0/64 base-partitions into one PSUM bank crash the HW stack) land
    in ONE [128,512] PSUM bank; exp/mask/normalize run as [128,512]-wide
    ACT/DVE ops (amortizes the ~150-250ns fixed per-instruction engine cost).
    One STREAM_TRANSPOSE per 4 heads: wei is 32-block-diagonal (4 batches of
    T=32), so the DVE's per-32x32-block transpose IS the full transpose.
  - Software pipelining: iteration c interleaves, at PE-unit granularity, the
    dense front of chunk c (qk+v), attention of chunk c-1 (out-matmuls trail
    their scores by two 4-head batches so the softmax DVE chain never
    head-of-line blocks the PE queue), and the projection of chunk c-2.
  - v-bias folded into the proj bias on host (softmax rows sum to 1).
  - attn^T staged through per-parity DRAM tensors (the only way to move head
    outputs from partitions 0:64 into the 128-partition proj K-layout).
  - Pass-boundary overlap: pools shared across both layer passes (slots free
    as soon as the last user finishes, not at pass end), per-chunk DRAM
    tensors for the inter-pass activations (no whole-tensor dependencies),
    and a quarter of W_ff1 preloaded into SBUF slack during pass 2.
"""
import sys

if "/opt/trn_rl_repo" not in sys.path:
    sys.path.insert(0, "/opt/trn_rl_repo")

from contextlib import ExitStack

import numpy as np

import concourse.bacc as bacc
import concourse.mybir as mybir
import concourse.tile as tile

F32 = mybir.dt.float32
BF16 = mybir.dt.bfloat16
F8E4 = mybir.dt.float8e4
P = 128

QK_FP8 = True      # qk matmuls in e4m3 DoubleRow (2 K-rows/cycle)
W8_SCALE = 64.0    # weight pre-scale into e4m3 sweet spot; descaled at evict

E = 1024
H = 16
D = 64
T = 32
HD = H * D  # 1024
FF = 4 * E  # 4096
N_CORES = 8
B_FULL = 1024
C = 512  # token chunk (columns per dense matmul)

AF = mybir.ActivationFunctionType
ALU = mybir.AluOpType

# bisection flags (all-True = fastest path)
FLAGS = {
    "big_transpose": True,   # one [128,512] STREAM_TRANSPOSE vs 4x[128,128]
    "scores_onebank": True,  # 4 scores MMs into one PSUM bank vs 4 banks
    "do_attn": True,         # emit attention (else memset ostrip)
    "batched_evict": True,   # one [64,512] ACT evict vs 4x[64,128]
}

# Head processing order: 4-head batches of UNIFORM parity, so the 4 scores
# matmuls sharing one PSUM bank all use the same tile_position (mixed 0/64
# row-positions into one bank crash the HW stack).  Consecutive batches
# ALTERNATE parity so their MMs land on disjoint PE row halves and overlap.
HEAD_BATCHES = [(0, 2, 4, 6), (1, 3, 5, 7), (8, 10, 12, 14), (9, 11, 13, 15)]

# FFN split-K fp8: first *_K8 contraction rows run e4m3 DoubleRow, the rest
# bf16.  Both halves' weights are pre-scaled x64 on host so the partial sums
# share one PSUM scale; the eviction's scale=1/64 descales for free.
# Error budget (CPU sim): err^2 ~ 0.0073^2 + f1*0.0184^2 + f2*0.0211^2.
FFN1_K8 = 512   # of E=1024   (f1=0.5)
FFN2_K8 = 512   # of FF=4096  (f2=0.125)


def _rd(ap):
    """DRAM [R, t] viewed as [128, R//128, t] (row r = k*128 + p)."""
    return ap.rearrange("(k p) t -> p k t", p=P)


def _chunk(ap_or_list, c):
    """Chunk c of an activation: per-chunk DRAM tensor list or one tensor."""
    if isinstance(ap_or_list, list):
        return _rd(ap_or_list[c])
    return _rd(ap_or_list)[:, :, slice(c * C, (c + 1) * C)]


def _layer_pass(tc, psums, pools, mask_sb, xin_d, Wqk_d, Wv_d, Wp_d,
                bqk_d, bp_d, stage_d, xout_d, tok,
                wqk_pre=None, prefetch_cb=None):
    """One attention layer: xout = xin + attn(xin) @ Wp + bp' (all bf16 I/O).

    Software-pipelined over chunks: iteration c emits, PE-interleaved at unit
    granularity, the dense front of chunk c (qk+v), the attention of chunk
    c-1, and the projection of chunk c-2.  Attention out-matmuls trail their
    scores by one 4-head batch so the softmax DVE chain never head-of-line
    blocks the PE queue.
    """
    nc = tc.nc
    mm_ps, s_ps_pool, o_ps_pool = psums
    n_chunks = tok // C
    n_grp = C // P  # groups of 4 batches per chunk
    scale = float(D) ** -0.5

    if True:
        wp = pools["wts"]
        if wqk_pre is not None:
            Wqk_sb = wqk_pre  # prefetched during the previous pass
        else:
            Wqk_sb = pools["wqk"].tile([P, E // P, 2 * HD],
                                       F8E4 if QK_FP8 else BF16, name="Wqk_sb")
        bqk_sb = wp.tile([P, 16], F32, name="bqk_sb")
        nc.sync.dma_start(bqk_sb[:], bqk_d[:])
        bp_sb = wp.tile([P, 8], F32, name="bp_sb")
        nc.sync.dma_start(bp_sb[:], bp_d[:])
        # Wv/Wp loads are deferred to just after the first x-chunk DMA so the
        # first qk matmuls aren't stuck behind them in the DMA queue.
        Wv_sb = wp.tile([P, HD // P, HD], BF16, name="Wv_sb")
        Wp_sb = wp.tile([P, HD // P, E], BF16, name="Wp_sb")

        x_pool = pools["x"]
        x8_pool = pools["x8"]
        qkT_pool = pools["qkT"]
        v_pool = pools["v"]
        w_pool = pools["w"]
        wT_pool = pools["wT"]
        st_pool = pools["st"]
        os_pool = pools["os"]
        aT_pool = pools["aT"]
        xo_pool = pools["xo"]

        x_tiles, x8_tiles, qkT_tiles, v_tiles, aT_tiles = {}, {}, {}, {}, {}

        def qk_unit(c, m):
            ps = mm_ps.tile([P, C], F32, name="mm")
            if QK_FP8:
                x8 = x8_tiles[c]
                for k in range(4):
                    nc.tensor.matmul(
                        ps[:],
                        lhsT=Wqk_sb[:, 2 * k:2 * k + 2, m * P:(m + 1) * P],
                        rhs=x8[:, 2 * k:2 * k + 2, :],
                        perf_mode=mybir.MatmulPerfMode.DoubleRow,
                        start=(k == 0), stop=(k == 3),
                    )
                nc.vector.tensor_scalar(
                    qkT_tiles[c][:, m, :], ps, 1.0 / W8_SCALE,
                    bqk_sb[:, m:m + 1], ALU.mult, ALU.add)
            else:
                for k in range(8):
                    nc.tensor.matmul(
                        ps[:], lhsT=Wqk_sb[:, k, m * P:(m + 1) * P],
                        rhs=x_tiles[c][:, k, :], start=(k == 0), stop=(k == 7),
                    )
                nc.vector.tensor_scalar_add(
                    qkT_tiles[c][:, m, :], ps, bqk_sb[:, m:m + 1])

        def v_unit(c, mt, n2):
            ps = mm_ps.tile([P, C], F32, name="mm")
            for k in range(8):
                nc.tensor.matmul(
                    ps[:], lhsT=x_tiles[c][:, k, mt * P:(mt + 1) * P],
                    rhs=Wv_sb[:, k, n2 * C:(n2 + 1) * C],
                    start=(k == 0), stop=(k == 7),
                )
            nc.vector.tensor_copy(
                out=v_tiles[c][:, mt, n2 * C:(n2 + 1) * C], in_=ps)

        def proj_unit(c, m):
            ps = mm_ps.tile([P, C], F32, name="mm")
            for k in range(8):
                nc.tensor.matmul(
                    ps[:], lhsT=Wp_sb[:, k, m * P:(m + 1) * P],
                    rhs=aT_tiles[c][:, k, :], start=(k == 0), stop=(k == 7),
                )
            xo = xo_tiles[c]
            # fused (ps + bias) + residual in one DVE pass
            nc.vector.scalar_tensor_tensor(
                out=xo[:, m, :], in0=ps, scalar=bp_sb[:, m:m + 1],
                in1=x_tiles[c][:, m, :], op0=ALU.add, op1=ALU.add)

        xo_tiles = {}

        def scores_front(c, g, hb):
            """4 scores MMs + softmax chain; returns weiT tile."""
            qkT = qkT_tiles[c]
            gc = slice(g * P, (g + 1) * P)
            heads = HEAD_BATCHES[hb]
            s_ps = s_ps_pool.tile([P, 4 * P], F32, name="s_ps")
            for j, h in enumerate(heads):
                r0 = D * (h % 2)
                nc.tensor.matmul(
                    s_ps[:, j * P:(j + 1) * P],
                    lhsT=qkT[r0:r0 + D, h // 2, gc],
                    rhs=qkT[r0:r0 + D, 8 + h // 2, gc],
                    start=True, stop=True,
                )
            e_sb = w_pool.tile([P, 4 * P], BF16, name="e_sb")
            nc.scalar.activation(e_sb, s_ps, AF.Exp, scale=scale)
            wei = w_pool.tile([P, 4 * P], BF16, name="wei")
            nc.vector.tensor_tensor(out=wei, in0=e_sb, in1=mask_sb, op=ALU.mult)
            sums = st_pool.tile([P, 4], F32, name="sums")
            nc.vector.tensor_reduce(
                out=sums, in_=wei.rearrange("p (j t) -> p j t", j=4),
                op=ALU.add, axis=mybir.AxisListType.X,
            )
            rcp = st_pool.tile([P, 4], F32, name="rcp")
            nc.vector.reciprocal(rcp, sums)
            nc.vector.tensor_tensor(
                out=wei.rearrange("p (j t) -> p j t", j=4),
                in0=wei.rearrange("p (j t) -> p j t", j=4),
                in1=rcp.unsqueeze(-1).broadcast_to([P, 4, P]),
                op=ALU.mult,
            )
            weiT = wT_pool.tile([P, 4 * P], BF16, name="weiT")
            nc.vector.transpose(weiT, wei)  # 32-blockwise == full T here
            return weiT

        def out_back(c, g, hb, weiT):
            """4 out MMs + strip eviction (+ stage DMA after last hb of g)."""
            heads = HEAD_BATCHES[hb]
            if hb == 0:
                ostrips[(c, g)] = os_pool.tile([D, H, P], BF16, name="ostrip")
            ostrip = ostrips[(c, g)]
            o_ps = o_ps_pool.tile([P, 4 * P], F32, name="o_ps")
            for j, h in enumerate(heads):
                nc.tensor.matmul(
                    o_ps[0:D, j * P:(j + 1) * P],
                    lhsT=v_tiles[c][:, g, h * D:(h + 1) * D],
                    rhs=weiT[:, j * P:(j + 1) * P],
                    start=True, stop=True,
                )
            nc.scalar.activation(
                ostrip[:, heads[0]:heads[-1] + 1:2, :],
                o_ps[0:D, :].rearrange("p (j t) -> p j t", j=4),
                AF.Copy,
            )
            if hb == 3:
                gc = slice(g * P, (g + 1) * P)
                nc.sync.dma_start(
                    stage_d[c % 2].rearrange("(h d) t -> d h t", d=D)[:, :, gc],
                    ostrip[:],
                )
                del ostrips[(c, g)]
                # read this group's columns straight back (partition shuffle);
                # per-group readback maximizes DMA lead time for proj
                if c not in aT_tiles:
                    aT_tiles[c] = aT_pool.tile([P, HD // P, C], BF16, name="aT")
                nc.sync.dma_start(
                    aT_tiles[c][:, :, gc], _rd(stage_d[c % 2])[:, :, gc])

        ostrips = {}

        def load_x(c):
            if c in x_tiles or c >= n_chunks:
                return
            x_sb = x_pool.tile([P, E // P, C], BF16, name="x_sb")
            nc.sync.dma_start(x_sb[:], _chunk(xin_d, c))
            x_tiles[c] = x_sb
            if c == 0:
                if wqk_pre is None:
                    nc.sync.dma_start(Wqk_sb[:, :, 0:HD], _rd(Wqk_d)[:, :, 0:HD])
                    nc.sync.dma_start(Wqk_sb[:, :, HD:2 * HD], _rd(Wqk_d)[:, :, HD:2 * HD])
                nc.sync.dma_start(Wv_sb[:], _rd(Wv_d))
                nc.sync.dma_start(Wp_sb[:], _rd(Wp_d))
            if QK_FP8:
                x8 = x8_pool.tile([P, E // P, C], F8E4, name="x8")
                for kq in range(4):
                    nc.vector.tensor_copy(
                        out=x8[:, 2 * kq:2 * kq + 2, :],
                        in_=x_sb[:, 2 * kq:2 * kq + 2, :])
                x8_tiles[c] = x8

        for c in range(n_chunks + 2):
            if c == 5 and prefetch_cb is not None:
                prefetch_cb()
            dense = []
            if c < n_chunks:
                load_x(c)
                load_x(c + 1)  # prefetch: next chunk's DMA a full iter early
                qkT_tiles[c] = qkT_pool.tile(
                    [P, 16, C], F8E4 if QK_FP8 else BF16, name="qkT")
                v_tiles[c] = v_pool.tile([P, n_grp, HD], BF16, name="V")
                dense += [(qk_unit, (c, m)) for m in range(16)]
                dense += [(v_unit, (c, mt, n2))
                          for mt in range(n_grp) for n2 in range(HD // C)]
            if c >= 2:
                xo_tiles[c - 2] = xo_pool.tile([P, E // P, C], BF16, name="xo")
                dense += [(proj_unit, (c - 2, m)) for m in range(8)]

            if 1 <= c <= n_chunks:
                ca = c - 1
                pend = []
                nd0, popped = len(dense), 0
                for ci, (g, hb) in enumerate(
                        (g, hb) for g in range(n_grp) for hb in range(4)):
                    weiT = scores_front(ca, g, hb)
                    # spread the dense units evenly across all 16 chains
                    want = (nd0 * (ci + 1) + 15) // 16 - popped
                    for _ in range(want):
                        if dense:
                            fn, args = dense.pop(0)
                            fn(*args)
                            popped += 1
                    if len(pend) >= 2:
                        out_back(*pend.pop(0))
                    pend.append((ca, g, hb, weiT))
                for fn, args in dense:
                    fn(*args)
                for pd in pend:
                    out_back(*pd)
                qkT_tiles.pop(ca)
                v_tiles.pop(ca)
                x8_tiles.pop(ca, None)
            else:
                for fn, args in dense:
                    fn(*args)

            if c >= 2:
                cp = c - 2
                nc.sync.dma_start(_chunk(xout_d, cp), xo_tiles.pop(cp)[:])
                x_tiles.pop(cp)
                aT_tiles.pop(cp)


def _ffn_pass(tc, psums, pools, preload, xin_d, Wff1b_d, Wff28_d, Wff2b_d,
              out_d, tok):
    """out = xin + relu(xin @ W1 + b1) @ W2 + b2   (out is fp32).

    Split-K fp8: contraction rows < FFN*_K8 run e4m3 DoubleRow, the rest
    bf16.  ALL weights are pre-scaled x64 on host so both halves' partial
    sums share one PSUM; evictions apply scale=1/64.
    """
    nc = tc.nc
    mm_ps, _, _ = psums
    n_chunks = tok // C
    k81 = FFN1_K8 // P   # fp8 k-groups in ffn1 (x side)
    k82 = FFN2_K8 // P   # fp8 m-tiles of h / k-groups in ffn2

    with ExitStack() as ctx:
        W18_sb, b1_sb, b2_sb = preload
        wp = ctx.enter_context(tc.tile_pool(name="fwts", bufs=1))
        x_pool = pools["x"]
        x8_pool = ctx.enter_context(tc.tile_pool(name="fx8_pool", bufs=2))
        h8_pool = ctx.enter_context(tc.tile_pool(name="fh8_pool", bufs=1))
        hb_pool = ctx.enter_context(tc.tile_pool(name="fhb_pool", bufs=1))
        o_pool = ctx.enter_context(tc.tile_pool(name="fo_pool", bufs=3))

        # first x chunk before the remaining big weight loads
        x0_sb = x_pool.tile([P, E // P, C], BF16, name="x_sb")
        nc.sync.dma_start(x0_sb[:], _chunk(xin_d, 0))
        W1b_sb = wp.tile([P, E // P - k81, FF], BF16, name="W1b_sb")
        nc.sync.dma_start(W1b_sb[:], _rd(Wff1b_d))
        # W2 is not needed until the first ffn2 (~30us in); load it behind W1
        W28_sb = wp.tile([P, k82, E], F8E4, name="W28_sb")
        nc.sync.dma_start(W28_sb[:], _rd(Wff28_d))
        W2b_sb = wp.tile([P, FF // P - k82, E], BF16, name="W2b_sb")
        nc.sync.dma_start(W2b_sb[:], _rd(Wff2b_d))

        fx_tiles = {}

        def fload_x(c):
            if c in fx_tiles or c >= n_chunks:
                return
            if c == 0:
                t = x0_sb
            else:
                t = x_pool.tile([P, E // P, C], BF16, name="x_sb")
                nc.sync.dma_start(t[:], _chunk(xin_d, c))
            x8 = x8_pool.tile([P, k81, C], F8E4, name="fx8")
            for kq in range(k81 // 2):
                nc.vector.tensor_copy(
                    out=x8[:, 2 * kq:2 * kq + 2, :],
                    in_=t[:, 2 * kq:2 * kq + 2, :])
            fx_tiles[c] = (t, x8)

        for c in range(n_chunks):
            cols = slice(c * C, (c + 1) * C)
            fload_x(c)
            fload_x(c + 1)
            x_sb, x8_sb = fx_tiles[c]

            hT8 = h8_pool.tile([P, k82, C], F8E4, name="hT8")
            hTb = hb_pool.tile([P, FF // P - k82, C], BF16, name="hTb")
            for m in range(FF // P):
                ps = mm_ps.tile([P, C], F32, name="mm")
                for kq in range(k81 // 2):
                    nc.tensor.matmul(
                        ps[:],
                        lhsT=W18_sb[:, 2 * kq:2 * kq + 2, m * P:(m + 1) * P],
                        rhs=x8_sb[:, 2 * kq:2 * kq + 2, :],
                        perf_mode=mybir.MatmulPerfMode.DoubleRow,
                        start=(kq == 0), stop=False,
                    )
                for k in range(k81, E // P):
                    nc.tensor.matmul(
                        ps[:], lhsT=W1b_sb[:, k - k81, m * P:(m + 1) * P],
                        rhs=x_sb[:, k, :], start=False, stop=(k == E // P - 1),
                    )
                hs = hT8[:, m, :] if m < k82 else hTb[:, m - k82, :]
                nc.scalar.activation(
                    hs, ps, AF.Relu, bias=b1_sb[:, m:m + 1],
                    scale=1.0 / W8_SCALE)

            for m in range(E // P):
                ps = mm_ps.tile([P, C], F32, name="mm")
                for kq in range(k82 // 2):
                    nc.tensor.matmul(
                        ps[:],
                        lhsT=W28_sb[:, 2 * kq:2 * kq + 2, m * P:(m + 1) * P],
                        rhs=hT8[:, 2 * kq:2 * kq + 2, :],
                        perf_mode=mybir.MatmulPerfMode.DoubleRow,
                        start=(kq == 0), stop=False,
                    )
                for k in range(k82, FF // P):
                    nc.tensor.matmul(
                        ps[:], lhsT=W2b_sb[:, k - k82, m * P:(m + 1) * P],
                        rhs=hTb[:, k - k82, :],
                        start=False, stop=(k == FF // P - 1),
                    )
                ot = o_pool.tile([P, C], F32, name="ot")
                nc.vector.tensor_scalar(
                    ot, ps, 1.0 / W8_SCALE, b2_sb[:, m:m + 1],
                    ALU.mult, ALU.add)
                nc.vector.tensor_tensor(
                    out=ot, in0=ot, in1=x_sb[:, m, :], op=ALU.add)
                nc.sync.dma_start(_rd(out_d)[:, m, cols], ot[:])


def build_kernel(b_shard):
    """Build the per-core Bass module for a batch shard of b_shard blocks."""
    tok = b_shard * T
    nc = bacc.Bacc(None, target_bir_lowering=False)
    with tile.TileContext(nc) as tc:
        with ExitStack() as ctx:
            dram = ctx.enter_context(tc.tile_pool(name="dram", bufs=1, space="DRAM"))

            def din(name, shape, dt=BF16):
                return dram.tile(shape, dt, kind="ExternalInput", uniquify=False, name=name)

            qk_dt = F8E4 if QK_FP8 else BF16
            xTb = din("xTb", [E, tok])
            Wqk1 = din("Wqk1", [E, 2 * HD], qk_dt); Wv1 = din("Wv1", [E, HD]); Wp1 = din("Wp1", [HD, E])
            Wqk2 = din("Wqk2", [E, 2 * HD], qk_dt); Wv2 = din("Wv2", [E, HD]); Wp2 = din("Wp2", [HD, E])
            Wff1_8 = din("Wff1_8", [FFN1_K8, FF], F8E4)
            Wff1_b = din("Wff1_b", [E - FFN1_K8, FF])
            Wff2_8 = din("Wff2_8", [FFN2_K8, E], F8E4)
            Wff2_b = din("Wff2_b", [FF - FFN2_K8, E])
            bqk1 = din("bqk1", [P, 16], F32); bp1 = din("bp1", [P, 8], F32)
            bqk2 = din("bqk2", [P, 16], F32); bp2 = din("bp2", [P, 8], F32)
            bff1 = din("bff1", [P, 32], F32); bff2 = din("bff2", [P, 8], F32)
            maskc = din("maskc", [P, 4 * P])

            outT = dram.tile([E, tok], F32, kind="ExternalOutput", uniquify=False, name="outT")

            n_ch = tok // C
            xT2b = [dram.tile([E, C], BF16, kind="Internal", uniquify=False, name=f"xT2b_{i}") for i in range(n_ch)]
            xT3b = [dram.tile([E, C], BF16, kind="Internal", uniquify=False, name=f"xT3b_{i}") for i in range(n_ch)]
            stg1 = [dram.tile([HD, C], BF16, kind="Internal", uniquify=False, name=f"stg1_{i}") for i in range(2)]
            stg2 = [dram.tile([HD, C], BF16, kind="Internal", uniquify=False, name=f"stg2_{i}") for i in range(2)]

            sb = 2 if FLAGS["scores_onebank"] else 1
            mm_ps = ctx.enter_context(tc.tile_pool(name="mm_ps", bufs=4, space="PSUM"))
            s_ps = ctx.enter_context(tc.tile_pool(name="s_ps", bufs=sb, space="PSUM"))
            o_ps = ctx.enter_context(tc.tile_pool(name="o_ps", bufs=sb, space="PSUM"))
            psums = (mm_ps, s_ps, o_ps)

            pools = {
                "x": ctx.enter_context(tc.tile_pool(name="x_pool", bufs=4)),
            }
            const_p = ctx.enter_context(tc.tile_pool(name="const", bufs=1))
            mask_sb = const_p.tile([P, 4 * P], BF16, name="mask_sb")
            nc.sync.dma_start(mask_sb[:], maskc[:])
            fpre = ctx.enter_context(tc.tile_pool(name="fwts_pre", bufs=1))

            with ExitStack() as lctx:
                for nm, bufs in (("wts", 1), ("wqk", 2), ("x8", 2), ("qkT", 2),
                                 ("v", 2), ("w", 3), ("wT", 3), ("st", 6),
                                 ("os", 3), ("aT", 2), ("xo", 2)):
                    pools[nm] = lctx.enter_context(
                        tc.tile_pool(name=nm + "_pool", bufs=bufs))

                box = {}

                def pre_l2():
                    # prefetch layer-2 qk weights mid-layer-1 (2nd wqk slot)
                    t = pools["wqk"].tile([P, E // P, 2 * HD],
                                          F8E4 if QK_FP8 else BF16,
                                          name="Wqk_sb")
                    nc.sync.dma_start(t[:, :, 0:HD], _rd(Wqk2)[:, :, 0:HD])
                    nc.sync.dma_start(t[:, :, HD:2 * HD], _rd(Wqk2)[:, :, HD:2 * HD])
                    box["wqk2"] = t

                def pre_ffn():
                    # prefetch the ffn1 fp8 weights + biases mid-layer-2
                    # (W2's fp8 part loads at FFN start; not needed till ~25us in)
                    W18_sb = fpre.tile([P, FFN1_K8 // P, FF], F8E4, name="W18_sb")
                    nc.sync.dma_start(W18_sb[:], _rd(Wff1_8))
                    b1_sb = fpre.tile([P, FF // P], F32, name="b1_sb")
                    nc.sync.dma_start(b1_sb[:], bff1[:])
                    b2_sb = fpre.tile([P, E // P], F32, name="b2_sb")
                    nc.sync.dma_start(b2_sb[:], bff2[:])
                    box["ffn"] = (W18_sb, b1_sb, b2_sb)

                _layer_pass(tc, psums, pools, mask_sb, xTb, Wqk1, Wv1, Wp1,
                            bqk1, bp1, stg1, xT2b, tok, prefetch_cb=pre_l2)
                _layer_pass(tc, psums, pools, mask_sb, xT2b, Wqk2, Wv2, Wp2,
                            bqk2, bp2, stg2, xT3b, tok,
                            wqk_pre=box["wqk2"], prefetch_cb=pre_ffn)
            _ffn_pass(tc, psums, pools, box["ffn"], xT3b, Wff1_b, Wff2_8,
                      Wff2_b, outT, tok)

    nc.compile()
    return nc


# --------------------------------------------------------------------------
# Host-side wrapper
# --------------------------------------------------------------------------

import ml_dtypes

BF16_NP = ml_dtypes.bfloat16


def _w_heads(W):
    """[H, E, D] -> [E, H*D] contiguous bf16 (col = 64h + d)."""
    return np.ascontiguousarray(
        np.transpose(np.asarray(W), (1, 0, 2)).reshape(E, HD).astype(BF16_NP)
    )


def _b_tile(b, n_po):
    """[Dim] -> [128, n_po] bias tile (row r = po*128 + pi)."""
    b = np.asarray(b, dtype=np.float32).reshape(n_po, P)
    return np.ascontiguousarray(b.T)


def _causal_mask_tile():
    """0/1 bf16 mask [128, 512]: 4-head tile of block-diag causal [128,128]."""
    m = np.zeros((P, P), dtype=np.float32)
    for p in range(P):
        blk, t = p // T, p % T
        m[p, blk * T: blk * T + t + 1] = 1.0
    return np.ascontiguousarray(np.tile(m, (1, 4)).astype(BF16_NP))


def make_in_maps(inputs, b_shard=B_FULL // N_CORES, n_cores=N_CORES):
    F8_NP = ml_dtypes.float8_e4m3
    x = np.asarray(inputs["x"], dtype=np.float32)

    def _q8(w):
        return np.ascontiguousarray(
            np.clip(w * W8_SCALE, -240, 240).astype(F8_NP))

    W1 = np.asarray(inputs["W_ff1"], np.float32)
    W2 = np.asarray(inputs["W_ff2"], np.float32)
    shared = {
        "maskc": _causal_mask_tile(),
        # split-K fp8: both halves x64 so PSUM partial sums share one scale
        "Wff1_8": _q8(W1[:FFN1_K8]),
        "Wff1_b": np.ascontiguousarray((W1[FFN1_K8:] * W8_SCALE).astype(BF16_NP)),
        "Wff2_8": _q8(W2[:FFN2_K8]),
        "Wff2_b": np.ascontiguousarray((W2[FFN2_K8:] * W8_SCALE).astype(BF16_NP)),
        "bff1": _b_tile(inputs["b_ff1"], 32),
        "bff2": _b_tile(inputs["b_ff2"], 8),
    }
    for li in ("1", "2"):
        Wq = _w_heads(inputs["Wq" + li])
        Wk = _w_heads(inputs["Wk" + li])
        wqk = np.ascontiguousarray(np.concatenate([Wq, Wk], axis=1))
        if QK_FP8:
            wqk = np.clip(wqk.astype(np.float32) * W8_SCALE, -240, 240).astype(F8_NP)
        shared["Wqk" + li] = wqk
        shared["Wv" + li] = _w_heads(inputs["Wv" + li])
        Wp = np.asarray(inputs["Wp" + li], np.float32)
        shared["Wp" + li] = np.ascontiguousarray(Wp.astype(BF16_NP))
        bq = np.asarray(inputs["bq" + li], np.float32).reshape(HD)
        bk = np.asarray(inputs["bk" + li], np.float32).reshape(HD)
        shared["bqk" + li] = _b_tile(np.concatenate([bq, bk]), 16)
        # fold v-bias through the projection:  bp' = bp + bv @ Wp
        bv = np.asarray(inputs["bv" + li], np.float32).reshape(HD)
        bp = np.asarray(inputs["bp" + li], np.float32) + bv @ Wp
        shared["bp" + li] = _b_tile(bp, 8)

    in_maps = []
    for c in range(n_cores):
        xs = x[c * b_shard:(c + 1) * b_shard].reshape(b_shard * T, E)
        m = dict(shared)
        m["xTb"] = np.ascontiguousarray(xs.T.astype(BF16_NP))
        in_maps.append(m)
    return in_maps


_NC_CACHE = {}


def kernel(**inputs) -> np.ndarray:
    from concourse.bass_utils import run_bass_kernel_spmd

    b_shard = B_FULL // N_CORES
    if b_shard not in _NC_CACHE:
        _NC_CACHE[b_shard] = build_kernel(b_shard)
    nc = _NC_CACHE[b_shard]

    in_maps = make_in_maps(inputs)
    res = run_bass_kernel_spmd(nc, in_maps, core_ids=list(range(N_CORES)))

    out = np.empty((B_FULL, T, E), dtype=np.float32)
    for c in range(N_CORES):
        outT = res.results[c]["outT"]  # [E, tok]
        out[c * b_shard:(c + 1) * b_shard] = outT.T.reshape(b_shard, T, E)
    return out



# revision 29
# speedup vs baseline: 1.1021x; 1.0257x over previous
"""Trainium2 Bass kernel for a 2x-MHA + FFN transformer block (fused pipeline).

Contract: kernel(**inputs) takes FULL unsharded inputs (numpy) and returns the
FULL output [1024, 32, 1024] float32.

Verified on HW: 1.940 ms (baseline 3.067 ms), absmax-rel err 7.8e-3.
v2: FFN split-K fp8 (rows < FFN*_K8 in e4m3 DoubleRow, both halves' weights
x64 so one PSUM scale; evict descales), layer-2/FFN weight prefetch,
alternating-parity head batches, fused proj eviction, drain rebalance.

Strategy:
  - Pure data-parallel over batch B=1024 across 8 NeuronCores (128 batches =
    4096 tokens per core). No collectives.
  - Transposed activations on device: xT [E, tokens] so every dense matmul is
    out = lhsT.T @ rhs with K (contraction) on partitions.
  - Three passes per core, each with its weights SBUF-RESIDENT, streaming the
    4096 tokens in 8 chunks of 512:
      pass 1: layer-1  qk -> attention -> proj(+residual)
      pass 2: layer-2  same
      pass 3: FFN      relu(x@W1+b1)@W2 + b2 (+residual), hT stays in SBUF
  - qk projections run in fp8 e4m3 DoubleRow (2 K-rows/cycle; weights
    pre-scaled x64 on host, descaled in the eviction).  fp8 elsewhere fails
    the 2e-2 gate (measured: ffn 2.7e-2, +v 2.0e-2); qk costs only +5e-4
    because softmax damps logit noise.
  - Residual spine is bf16 (error budget allows; halves DMA + SBUF).
  - Attention is batched 4 heads per step: scores for 4 SAME-PARITY heads
    (mixed 0/64 base-partitions into one PSUM bank crash the HW stack) land
    in ONE [128,512] PSUM bank; exp/mask/normalize run as [128,512]-wide
    ACT/DVE ops (amortizes the ~150-250ns fixed per-instruction engine cost).
    One STREAM_TRANSPOSE per 4 heads: wei is 32-block-diagonal (4 batches of
    T=32), so the DVE's per-32x32-block transpose IS the full transpose.
  - Software pipelining: iteration c interleaves, at PE-unit granularity, the
    dense front of chunk c (qk+v), attention of chunk c-1 (out-matmuls trail
    their scores by two 4-head batches so the softmax DVE chain never
    head-of-line blocks the PE queue), and the projection of chunk c-2.
  - v-bias folded into the proj bias on host (softmax rows sum to 1).
  - attn^T staged through per-parity DRAM tensors (the only way to move head
    outputs from partitions 0:64 into the 128-partition proj K-layout).
  - Pass-boundary overlap: pools shared across both layer passes (slots free
    as soon as the last user finishes, not at pass end), per-chunk DRAM
    tensors for the inter-pass activations (no whole-tensor dependencies),
    and a quarter of W_ff1 preloaded into SBUF slack during pass 2.
"""
import sys

if "/opt/trn_rl_repo" not in sys.path:
    sys.path.insert(0, "/opt/trn_rl_repo")

from contextlib import ExitStack

import numpy as np

import concourse.bacc as bacc
import concourse.mybir as mybir
import concourse.tile as tile

F32 = mybir.dt.float32
BF16 = mybir.dt.bfloat16
F8E4 = mybir.dt.float8e4
P = 128

QK_FP8 = True      # qk matmuls in e4m3 DoubleRow (2 K-rows/cycle)
W8_SCALE = 64.0    # weight pre-scale into e4m3 sweet spot; descaled at evict

E = 1024
H = 16
D = 64
T = 32
HD = H * D  # 1024
FF = 4 * E  # 4096
N_CORES = 8
B_FULL = 1024
C = 512  # token chunk (columns per dense matmul)

AF = mybir.ActivationFunctionType
ALU = mybir.AluOpType

# bisection flags (all-True = fastest path)
FLAGS = {
    "big_transpose": True,   # one [128,512] STREAM_TRANSPOSE vs 4x[128,128]
    "scores_onebank": True,  # 4 scores MMs into one PSUM bank vs 4 banks
    "do_attn": True,         # emit attention (else memset ostrip)
    "batched_evict": True,   # one [64,512] ACT evict vs 4x[64,128]
}

# Head processing order: 4-head batches of UNIFORM parity, so the 4 scores
# matmuls sharing one PSUM bank all use the same tile_position (mixed 0/64
# row-positions into one bank crash the HW stack).  Consecutive batches
# ALTERNATE parity so their MMs land on disjoint PE row halves and overlap.
HEAD_BATCHES = [(0, 2, 4, 6), (1, 3, 5, 7), (8, 10, 12, 14), (9, 11, 13, 15)]

# FFN split-K fp8: first *_K8 contraction rows run e4m3 DoubleRow, the rest
# bf16.  Both halves' weights are pre-scaled x64 on host so the partial sums
# share one PSUM scale; the eviction's scale=1/64 descales for free.
# Error budget (CPU sim): err^2 ~ 0.0073^2 + f1*0.0184^2 + f2*0.0211^2.
FFN1_K8 = 512   # of E=1024   (f1=0.5)
FFN2_K8 = 512   # of FF=4096  (f2=0.125)


def _rd(ap):
    """DRAM [R, t] viewed as [128, R//128, t] (row r = k*128 + p)."""
    return ap.rearrange("(k p) t -> p k t", p=P)


def _chunk(ap_or_list, c):
    """Chunk c of an activation: per-chunk DRAM tensor list or one tensor."""
    if isinstance(ap_or_list, list):
        return _rd(ap_or_list[c])
    return _rd(ap_or_list)[:, :, slice(c * C, (c + 1) * C)]


def _layer_pass(tc, psums, pools, mask_sb, xin_d, Wqk_d, Wv_d, Wp_d,
                bqk_d, bp_d, xout_d, tok,
                wqk_pre=None, prefetch_cb=None):
    """One attention layer: xout = xin + attn(xin) @ Wp + bp' (all bf16 I/O).

    Software-pipelined over chunks: iteration c emits, PE-interleaved at unit
    granularity, the dense front of chunk c (qk+v), the attention of chunk
    c-1, and the projection of chunk c-2.  Attention out-matmuls trail their
    scores by one 4-head batch so the softmax DVE chain never head-of-line
    blocks the PE queue.
    """
    nc = tc.nc
    mm_ps, s_ps_pool, o_ps_pool = psums
    n_chunks = tok // C
    n_grp = C // P  # groups of 4 batches per chunk
    scale = float(D) ** -0.5

    if True:
        wp = pools["wts"]
        if wqk_pre is not None:
            Wqk_sb = wqk_pre  # prefetched during the previous pass
        else:
            Wqk_sb = pools["wqk"].tile([P, E // P, 2 * HD],
                                       F8E4 if QK_FP8 else BF16, name="Wqk_sb")
        bqk_sb = wp.tile([P, 16], F32, name="bqk_sb")
        nc.sync.dma_start(bqk_sb[:], bqk_d[:])
        bp_sb = wp.tile([P, 8], F32, name="bp_sb")
        nc.sync.dma_start(bp_sb[:], bp_d[:])
        # Wv/Wp loads are deferred to just after the first x-chunk DMA so the
        # first qk matmuls aren't stuck behind them in the DMA queue.
        Wv_sb = wp.tile([P, HD // P, HD], BF16, name="Wv_sb")
        Wp_sb = wp.tile([P, HD // P, E], BF16, name="Wp_sb")

        x_pool = pools["x"]
        x8_pool = pools["x8"]
        qkT_pool = pools["qkT"]
        v_pool = pools["v"]
        w_pool = pools["w"]
        wT_pool = pools["wT"]
        st_pool = pools["st"]
        aT_pool = pools["aT"]
        xo_pool = pools["xo"]

        x_tiles, x8_tiles, qkT_tiles, v_tiles, aT_tiles = {}, {}, {}, {}, {}

        def qk_unit(c, m):
            ps = mm_ps.tile([P, C], F32, name="mm")
            if QK_FP8:
                x8 = x8_tiles[c]
                for k in range(4):
                    nc.tensor.matmul(
                        ps[:],
                        lhsT=Wqk_sb[:, 2 * k:2 * k + 2, m * P:(m + 1) * P],
                        rhs=x8[:, 2 * k:2 * k + 2, :],
                        perf_mode=mybir.MatmulPerfMode.DoubleRow,
                        start=(k == 0), stop=(k == 3),
                    )
                nc.vector.tensor_scalar(
                    qkT_tiles[c][:, m, :], ps, 1.0 / W8_SCALE,
                    bqk_sb[:, m:m + 1], ALU.mult, ALU.add)
            else:
                for k in range(8):
                    nc.tensor.matmul(
                        ps[:], lhsT=Wqk_sb[:, k, m * P:(m + 1) * P],
                        rhs=x_tiles[c][:, k, :], start=(k == 0), stop=(k == 7),
                    )
                nc.vector.tensor_scalar_add(
                    qkT_tiles[c][:, m, :], ps, bqk_sb[:, m:m + 1])

        def v_unit(c, mt, n2):
            ps = mm_ps.tile([P, C], F32, name="mm")
            for k in range(8):
                nc.tensor.matmul(
                    ps[:], lhsT=x_tiles[c][:, k, mt * P:(mt + 1) * P],
                    rhs=Wv_sb[:, k, n2 * C:(n2 + 1) * C],
                    start=(k == 0), stop=(k == 7),
                )
            nc.vector.tensor_copy(
                out=v_tiles[c][:, mt, n2 * C:(n2 + 1) * C], in_=ps)

        def proj_unit(c, m):
            ps = mm_ps.tile([P, C], F32, name="mm")
            for k in range(8):
                nc.tensor.matmul(
                    ps[:], lhsT=Wp_sb[:, k, m * P:(m + 1) * P],
                    rhs=aT_tiles[c][:, k, :], start=(k == 0), stop=(k == 7),
                )
            xo = xo_tiles[c]
            # fused (ps + bias) + residual in one DVE pass
            nc.vector.scalar_tensor_tensor(
                out=xo[:, m, :], in0=ps, scalar=bp_sb[:, m:m + 1],
                in1=x_tiles[c][:, m, :], op0=ALU.add, op1=ALU.add)

        xo_tiles = {}

        def scores_front(c, g, hb):
            """4 scores MMs + softmax chain; returns weiT tile."""
            qkT = qkT_tiles[c]
            gc = slice(g * P, (g + 1) * P)
            heads = HEAD_BATCHES[hb]
            s_ps = s_ps_pool.tile([P, 4 * P], F32, name="s_ps")
            for j, h in enumerate(heads):
                r0 = D * (h % 2)
                nc.tensor.matmul(
                    s_ps[:, j * P:(j + 1) * P],
                    lhsT=qkT[r0:r0 + D, h // 2, gc],
                    rhs=qkT[r0:r0 + D, 8 + h // 2, gc],
                    start=True, stop=True,
                )
            e_sb = w_pool.tile([P, 4 * P], BF16, name="e_sb")
            nc.scalar.activation(e_sb, s_ps, AF.Exp, scale=scale)
            wei = w_pool.tile([P, 4 * P], BF16, name="wei")
            nc.vector.tensor_tensor(out=wei, in0=e_sb, in1=mask_sb, op=ALU.mult)
            sums = st_pool.tile([P, 4], F32, name="sums")
            nc.vector.tensor_reduce(
                out=sums, in_=wei.rearrange("p (j t) -> p j t", j=4),
                op=ALU.add, axis=mybir.AxisListType.X,
            )
            rcp = st_pool.tile([P, 4], F32, name="rcp")
            nc.vector.reciprocal(rcp, sums)
            nc.vector.tensor_tensor(
                out=wei.rearrange("p (j t) -> p j t", j=4),
                in0=wei.rearrange("p (j t) -> p j t", j=4),
                in1=rcp.unsqueeze(-1).broadcast_to([P, 4, P]),
                op=ALU.mult,
            )
            weiT = wT_pool.tile([P, 4 * P], BF16, name="weiT")
            nc.vector.transpose(weiT, wei)  # 32-blockwise == full T here
            return weiT

        def out_back(c, g, hbe, weiT_e, weiT_o):
            """8 out MMs for a batch PAIR (even-parity hbe, odd hbe+1) into one
            bank: even heads -> psum partitions 0:64, odd heads -> 64:128 via
            col tiling.  Each head pair (2k, 2k+1) then IS aT feature block k,
            evicted straight to SBUF -- no DRAM stage roundtrip."""
            heads_e = HEAD_BATCHES[hbe]
            gc = slice(g * P, (g + 1) * P)
            o_ps = o_ps_pool.tile([P, 4 * P], F32, name="o_ps")
            for j, he in enumerate(heads_e):
                ho = he + 1
                nc.tensor.matmul(
                    o_ps[0:D, j * P:(j + 1) * P],
                    lhsT=v_tiles[c][:, g, he * D:(he + 1) * D],
                    rhs=weiT_e[:, j * P:(j + 1) * P],
                    start=True, stop=True,
                )
                nc.tensor.matmul(
                    o_ps[D:P, j * P:(j + 1) * P],
                    lhsT=v_tiles[c][:, g, ho * D:(ho + 1) * D],
                    rhs=weiT_o[:, j * P:(j + 1) * P],
                    start=True, stop=True, tile_position=(0, D),
                )
            if c not in aT_tiles:
                aT_tiles[c] = aT_pool.tile([P, HD // P, C], BF16, name="aT")
            k0 = heads_e[0] // 2  # feature block of the first pair
            nc.scalar.activation(
                aT_tiles[c][:, k0:k0 + 4, gc],
                o_ps[:].rearrange("p (j t) -> p j t", j=4),
                AF.Copy,
            )

        def load_x(c):
            if c in x_tiles or c >= n_chunks:
                return
            x_sb = x_pool.tile([P, E // P, C], BF16, name="x_sb")
            nc.sync.dma_start(x_sb[:], _chunk(xin_d, c))
            x_tiles[c] = x_sb
            if c == 0:
                if wqk_pre is None:
                    nc.sync.dma_start(Wqk_sb[:, :, 0:HD], _rd(Wqk_d)[:, :, 0:HD])
                    nc.sync.dma_start(Wqk_sb[:, :, HD:2 * HD], _rd(Wqk_d)[:, :, HD:2 * HD])
                nc.sync.dma_start(Wv_sb[:], _rd(Wv_d))
                nc.sync.dma_start(Wp_sb[:], _rd(Wp_d))
            if QK_FP8:
                x8 = x8_pool.tile([P, E // P, C], F8E4, name="x8")
                for kq in range(4):
                    nc.vector.tensor_copy(
                        out=x8[:, 2 * kq:2 * kq + 2, :],
                        in_=x_sb[:, 2 * kq:2 * kq + 2, :])
                x8_tiles[c] = x8

        for c in range(n_chunks + 2):
            if c == 5 and prefetch_cb is not None:
                prefetch_cb()
            dense = []
            if c < n_chunks:
                load_x(c)
                load_x(c + 1)  # prefetch: next chunk's DMA a full iter early
                qkT_tiles[c] = qkT_pool.tile(
                    [P, 16, C], F8E4 if QK_FP8 else BF16, name="qkT")
                v_tiles[c] = v_pool.tile([P, n_grp, HD], BF16, name="V")
                dense += [(qk_unit, (c, m)) for m in range(16)]
                dense += [(v_unit, (c, mt, n2))
                          for mt in range(n_grp) for n2 in range(HD // C)]
            if c >= 2:
                xo_tiles[c - 2] = xo_pool.tile([P, E // P, C], BF16, name="xo")
                dense += [(proj_unit, (c - 2, m)) for m in range(8)]

            if 1 <= c <= n_chunks:
                ca = c - 1
                pend = []
                weiT_even = None
                nd0, popped = len(dense), 0
                for ci, (g, hb) in enumerate(
                        (g, hb) for g in range(n_grp) for hb in range(4)):
                    weiT = scores_front(ca, g, hb)
                    # spread the dense units evenly across all 16 chains
                    want = (nd0 * (ci + 1) + 15) // 16 - popped
                    for _ in range(want):
                        if dense:
                            fn, args = dense.pop(0)
                            fn(*args)
                            popped += 1
                    if hb % 2:
                        if pend:
                            out_back(*pend.pop(0))
                        pend.append((ca, g, hb - 1, weiT_even, weiT))
                    else:
                        weiT_even = weiT
                for fn, args in dense:
                    fn(*args)
                for pd in pend:
                    out_back(*pd)
                qkT_tiles.pop(ca)
                v_tiles.pop(ca)
                x8_tiles.pop(ca, None)
            else:
                for fn, args in dense:
                    fn(*args)

            if c >= 2:
                cp = c - 2
                nc.sync.dma_start(_chunk(xout_d, cp), xo_tiles.pop(cp)[:])
                x_tiles.pop(cp)
                aT_tiles.pop(cp)


def _ffn_pass(tc, psums, pools, preload, xin_d, Wff1b_d, Wff28_d, Wff2b_d,
              out_d, tok):
    """out = xin + relu(xin @ W1 + b1) @ W2 + b2   (out is fp32).

    Split-K fp8: contraction rows < FFN*_K8 run e4m3 DoubleRow, the rest
    bf16.  ALL weights are pre-scaled x64 on host so both halves' partial
    sums share one PSUM; evictions apply scale=1/64.
    """
    nc = tc.nc
    mm_ps, _, _ = psums
    n_chunks = tok // C
    k81 = FFN1_K8 // P   # fp8 k-groups in ffn1 (x side)
    k82 = FFN2_K8 // P   # fp8 m-tiles of h / k-groups in ffn2

    with ExitStack() as ctx:
        W18_sb, b1_sb, b2_sb = preload
        wp = ctx.enter_context(tc.tile_pool(name="fwts", bufs=1))
        x_pool = pools["x"]
        x8_pool = ctx.enter_context(tc.tile_pool(name="fx8_pool", bufs=2))
        h8_pool = ctx.enter_context(tc.tile_pool(name="fh8_pool", bufs=1))
        hb_pool = ctx.enter_context(tc.tile_pool(name="fhb_pool", bufs=1))
        o_pool = ctx.enter_context(tc.tile_pool(name="fo_pool", bufs=3))

        # first x chunk before the remaining big weight loads
        x0_sb = x_pool.tile([P, E // P, C], BF16, name="x_sb")
        nc.sync.dma_start(x0_sb[:], _chunk(xin_d, 0))
        # column-chunked loads so the first m-tiles' matmuls dep only on
        # their own region, not the whole multi-MB transfer
        W1b_sb = wp.tile([P, E // P - k81, FF], BF16, name="W1b_sb")
        for q in range(4):
            cq = slice(q * FF // 4, (q + 1) * FF // 4)
            nc.sync.dma_start(W1b_sb[:, :, cq], _rd(Wff1b_d)[:, :, cq])
        # W2 is not needed until the first ffn2 (~30us in); load it behind W1
        W28_sb = wp.tile([P, k82, E], F8E4, name="W28_sb")
        nc.sync.dma_start(W28_sb[:], _rd(Wff28_d))
        W2b_sb = wp.tile([P, FF // P - k82, E], BF16, name="W2b_sb")
        for q in range(4):
            cq = slice(q * E // 4, (q + 1) * E // 4)
            nc.sync.dma_start(W2b_sb[:, :, cq], _rd(Wff2b_d)[:, :, cq])

        fx_tiles = {}

        def fload_x(c):
            if c in fx_tiles or c >= n_chunks:
                return
            if c == 0:
                t = x0_sb
            else:
                t = x_pool.tile([P, E // P, C], BF16, name="x_sb")
                nc.sync.dma_start(t[:], _chunk(xin_d, c))
            x8 = x8_pool.tile([P, k81, C], F8E4, name="fx8")
            for kq in range(k81 // 2):
                nc.vector.tensor_copy(
                    out=x8[:, 2 * kq:2 * kq + 2, :],
                    in_=t[:, 2 * kq:2 * kq + 2, :])
            fx_tiles[c] = (t, x8)

        for c in range(n_chunks):
            cols = slice(c * C, (c + 1) * C)
            fload_x(c)
            fload_x(c + 1)
            x_sb, x8_sb = fx_tiles[c]

            hT8 = h8_pool.tile([P, k82, C], F8E4, name="hT8")
            hTb = hb_pool.tile([P, FF // P - k82, C], BF16, name="hTb")
            for m in range(FF // P):
                ps = mm_ps.tile([P, C], F32, name="mm")
                for kq in range(k81 // 2):
                    nc.tensor.matmul(
                        ps[:],
                        lhsT=W18_sb[:, 2 * kq:2 * kq + 2, m * P:(m + 1) * P],
                        rhs=x8_sb[:, 2 * kq:2 * kq + 2, :],
                        perf_mode=mybir.MatmulPerfMode.DoubleRow,
                        start=(kq == 0), stop=False,
                    )
                for k in range(k81, E // P):
                    nc.tensor.matmul(
                        ps[:], lhsT=W1b_sb[:, k - k81, m * P:(m + 1) * P],
                        rhs=x_sb[:, k, :], start=False, stop=(k == E // P - 1),
                    )
                hs = hT8[:, m, :] if m < k82 else hTb[:, m - k82, :]
                nc.scalar.activation(
                    hs, ps, AF.Relu, bias=b1_sb[:, m:m + 1],
                    scale=1.0 / W8_SCALE)

            for m in range(E // P):
                ps = mm_ps.tile([P, C], F32, name="mm")
                for kq in range(k82 // 2):
                    nc.tensor.matmul(
                        ps[:],
                        lhsT=W28_sb[:, 2 * kq:2 * kq + 2, m * P:(m + 1) * P],
                        rhs=hT8[:, 2 * kq:2 * kq + 2, :],
                        perf_mode=mybir.MatmulPerfMode.DoubleRow,
                        start=(kq == 0), stop=False,
                    )
                for k in range(k82, FF // P):
                    nc.tensor.matmul(
                        ps[:], lhsT=W2b_sb[:, k - k82, m * P:(m + 1) * P],
                        rhs=hTb[:, k - k82, :],
                        start=False, stop=(k == FF // P - 1),
                    )
                ot = o_pool.tile([P, C], F32, name="ot")
                nc.vector.tensor_scalar(
                    ot, ps, 1.0 / W8_SCALE, b2_sb[:, m:m + 1],
                    ALU.mult, ALU.add)
                nc.vector.tensor_tensor(
                    out=ot, in0=ot, in1=x_sb[:, m, :], op=ALU.add)
                nc.sync.dma_start(_rd(out_d)[:, m, cols], ot[:])


def build_kernel(b_shard):
    """Build the per-core Bass module for a batch shard of b_shard blocks."""
    tok = b_shard * T
    nc = bacc.Bacc(None, target_bir_lowering=False)
    with tile.TileContext(nc) as tc:
        with ExitStack() as ctx:
            dram = ctx.enter_context(tc.tile_pool(name="dram", bufs=1, space="DRAM"))

            def din(name, shape, dt=BF16):
                return dram.tile(shape, dt, kind="ExternalInput", uniquify=False, name=name)

            qk_dt = F8E4 if QK_FP8 else BF16
            xTb = din("xTb", [E, tok])
            Wqk1 = din("Wqk1", [E, 2 * HD], qk_dt); Wv1 = din("Wv1", [E, HD]); Wp1 = din("Wp1", [HD, E])
            Wqk2 = din("Wqk2", [E, 2 * HD], qk_dt); Wv2 = din("Wv2", [E, HD]); Wp2 = din("Wp2", [HD, E])
            Wff1_8 = din("Wff1_8", [FFN1_K8, FF], F8E4)
            Wff1_b = din("Wff1_b", [E - FFN1_K8, FF])
            Wff2_8 = din("Wff2_8", [FFN2_K8, E], F8E4)
            Wff2_b = din("Wff2_b", [FF - FFN2_K8, E])
            bqk1 = din("bqk1", [P, 16], F32); bp1 = din("bp1", [P, 8], F32)
            bqk2 = din("bqk2", [P, 16], F32); bp2 = din("bp2", [P, 8], F32)
            bff1 = din("bff1", [P, 32], F32); bff2 = din("bff2", [P, 8], F32)
            maskc = din("maskc", [P, 4 * P])

            outT = dram.tile([E, tok], F32, kind="ExternalOutput", uniquify=False, name="outT")

            n_ch = tok // C
            xT2b = [dram.tile([E, C], BF16, kind="Internal", uniquify=False, name=f"xT2b_{i}") for i in range(n_ch)]
            xT3b = [dram.tile([E, C], BF16, kind="Internal", uniquify=False, name=f"xT3b_{i}") for i in range(n_ch)]

            sb = 2 if FLAGS["scores_onebank"] else 1
            mm_ps = ctx.enter_context(tc.tile_pool(name="mm_ps", bufs=4, space="PSUM"))
            s_ps = ctx.enter_context(tc.tile_pool(name="s_ps", bufs=sb, space="PSUM"))
            o_ps = ctx.enter_context(tc.tile_pool(name="o_ps", bufs=sb, space="PSUM"))
            psums = (mm_ps, s_ps, o_ps)

            pools = {
                "x": ctx.enter_context(tc.tile_pool(name="x_pool", bufs=4)),
            }
            const_p = ctx.enter_context(tc.tile_pool(name="const", bufs=1))
            mask_sb = const_p.tile([P, 4 * P], BF16, name="mask_sb")
            nc.sync.dma_start(mask_sb[:], maskc[:])
            fpre = ctx.enter_context(tc.tile_pool(name="fwts_pre", bufs=1))

            with ExitStack() as lctx:
                for nm, bufs in (("wts", 1), ("wqk", 2), ("x8", 2), ("qkT", 2),
                                 ("v", 2), ("w", 3), ("wT", 4), ("st", 6),
                                 ("aT", 2), ("xo", 2)):
                    pools[nm] = lctx.enter_context(
                        tc.tile_pool(name=nm + "_pool", bufs=bufs))

                box = {}

                def pre_l2():
                    # prefetch layer-2 qk weights mid-layer-1 (2nd wqk slot)
                    t = pools["wqk"].tile([P, E // P, 2 * HD],
                                          F8E4 if QK_FP8 else BF16,
                                          name="Wqk_sb")
                    nc.sync.dma_start(t[:, :, 0:HD], _rd(Wqk2)[:, :, 0:HD])
                    nc.sync.dma_start(t[:, :, HD:2 * HD], _rd(Wqk2)[:, :, HD:2 * HD])
                    box["wqk2"] = t

                def pre_ffn():
                    # prefetch the ffn1 fp8 weights + biases mid-layer-2
                    # (W2's fp8 part loads at FFN start; not needed till ~25us in)
                    W18_sb = fpre.tile([P, FFN1_K8 // P, FF], F8E4, name="W18_sb")
                    nc.sync.dma_start(W18_sb[:], _rd(Wff1_8))
                    b1_sb = fpre.tile([P, FF // P], F32, name="b1_sb")
                    nc.sync.dma_start(b1_sb[:], bff1[:])
                    b2_sb = fpre.tile([P, E // P], F32, name="b2_sb")
                    nc.sync.dma_start(b2_sb[:], bff2[:])
                    box["ffn"] = (W18_sb, b1_sb, b2_sb)

                _layer_pass(tc, psums, pools, mask_sb, xTb, Wqk1, Wv1, Wp1,
                            bqk1, bp1, xT2b, tok, prefetch_cb=pre_l2)
                _layer_pass(tc, psums, pools, mask_sb, xT2b, Wqk2, Wv2, Wp2,
                            bqk2, bp2, xT3b, tok,
                            wqk_pre=box["wqk2"], prefetch_cb=pre_ffn)
            _ffn_pass(tc, psums, pools, box["ffn"], xT3b, Wff1_b, Wff2_8,
                      Wff2_b, outT, tok)

    nc.compile()
    return nc


# --------------------------------------------------------------------------
# Host-side wrapper
# --------------------------------------------------------------------------

import ml_dtypes

BF16_NP = ml_dtypes.bfloat16


def _w_heads(W):
    """[H, E, D] -> [E, H*D] contiguous bf16 (col = 64h + d)."""
    return np.ascontiguousarray(
        np.transpose(np.asarray(W), (1, 0, 2)).reshape(E, HD).astype(BF16_NP)
    )


def _b_tile(b, n_po):
    """[Dim] -> [128, n_po] bias tile (row r = po*128 + pi)."""
    b = np.asarray(b, dtype=np.float32).reshape(n_po, P)
    return np.ascontiguousarray(b.T)


def _causal_mask_tile():
    """0/1 bf16 mask [128, 512]: 4-head tile of block-diag causal [128,128]."""
    m = np.zeros((P, P), dtype=np.float32)
    for p in range(P):
        blk, t = p // T, p % T
        m[p, blk * T: blk * T + t + 1] = 1.0
    return np.ascontiguousarray(np.tile(m, (1, 4)).astype(BF16_NP))


def make_in_maps(inputs, b_shard=B_FULL // N_CORES, n_cores=N_CORES):
    F8_NP = ml_dtypes.float8_e4m3
    x = np.asarray(inputs["x"], dtype=np.float32)

    def _q8(w):
        return np.ascontiguousarray(
            np.clip(w * W8_SCALE, -240, 240).astype(F8_NP))

    W1 = np.asarray(inputs["W_ff1"], np.float32)
    W2 = np.asarray(inputs["W_ff2"], np.float32)
    shared = {
        "maskc": _causal_mask_tile(),
        # split-K fp8: both halves x64 so PSUM partial sums share one scale
        "Wff1_8": _q8(W1[:FFN1_K8]),
        "Wff1_b": np.ascontiguousarray((W1[FFN1_K8:] * W8_SCALE).astype(BF16_NP)),
        "Wff2_8": _q8(W2[:FFN2_K8]),
        "Wff2_b": np.ascontiguousarray((W2[FFN2_K8:] * W8_SCALE).astype(BF16_NP)),
        "bff1": _b_tile(inputs["b_ff1"], 32),
        "bff2": _b_tile(inputs["b_ff2"], 8),
    }
    for li in ("1", "2"):
        Wq = _w_heads(inputs["Wq" + li])
        Wk = _w_heads(inputs["Wk" + li])
        wqk = np.ascontiguousarray(np.concatenate([Wq, Wk], axis=1))
        if QK_FP8:
            wqk = np.clip(wqk.astype(np.float32) * W8_SCALE, -240, 240).astype(F8_NP)
        shared["Wqk" + li] = wqk
        shared["Wv" + li] = _w_heads(inputs["Wv" + li])
        Wp = np.asarray(inputs["Wp" + li], np.float32)
        shared["Wp" + li] = np.ascontiguousarray(Wp.astype(BF16_NP))
        bq = np.asarray(inputs["bq" + li], np.float32).reshape(HD)
        bk = np.asarray(inputs["bk" + li], np.float32).reshape(HD)
        shared["bqk" + li] = _b_tile(np.concatenate([bq, bk]), 16)
        # fold v-bias through the projection:  bp' = bp + bv @ Wp
        bv = np.asarray(inputs["bv" + li], np.float32).reshape(HD)
        bp = np.asarray(inputs["bp" + li], np.float32) + bv @ Wp
        shared["bp" + li] = _b_tile(bp, 8)

    in_maps = []
    for c in range(n_cores):
        xs = x[c * b_shard:(c + 1) * b_shard].reshape(b_shard * T, E)
        m = dict(shared)
        m["xTb"] = np.ascontiguousarray(xs.T.astype(BF16_NP))
        in_maps.append(m)
    return in_maps


_NC_CACHE = {}


def kernel(**inputs) -> np.ndarray:
    from concourse.bass_utils import run_bass_kernel_spmd

    b_shard = B_FULL // N_CORES
    if b_shard not in _NC_CACHE:
        _NC_CACHE[b_shard] = build_kernel(b_shard)
    nc = _NC_CACHE[b_shard]

    in_maps = make_in_maps(inputs)
    res = run_bass_kernel_spmd(nc, in_maps, core_ids=list(range(N_CORES)))

    out = np.empty((B_FULL, T, E), dtype=np.float32)
    for c in range(N_CORES):
        outT = res.results[c]["outT"]  # [E, tok]
        out[c * b_shard:(c + 1) * b_shard] = outT.T.reshape(b_shard, T, E)
    return out



# revision 39
# speedup vs baseline: 1.1340x; 1.0290x over previous
"""Trainium2 Bass kernel for a 2x-MHA + FFN transformer block (fused pipeline).

Contract: kernel(**inputs) takes FULL unsharded inputs (numpy) and returns the
FULL output [1024, 32, 1024] float32.

Verified on HW: 1.940 ms (baseline 3.067 ms), absmax-rel err 7.8e-3.
v2: FFN split-K fp8 (rows < FFN*_K8 in e4m3 DoubleRow, both halves' weights
x64 so one PSUM scale; evict descales), layer-2/FFN weight prefetch,
alternating-parity head batches, fused proj eviction, drain rebalance.

Strategy:
  - Pure data-parallel over batch B=1024 across 8 NeuronCores (128 batches =
    4096 tokens per core). No collectives.
  - Transposed activations on device: xT [E, tokens] so every dense matmul is
    out = lhsT.T @ rhs with K (contraction) on partitions.
  - Three passes per core, each with its weights SBUF-RESIDENT, streaming the
    4096 tokens in 8 chunks of 512:
      pass 1: layer-1  qk -> attention -> proj(+residual)
      pass 2: layer-2  same
      pass 3: FFN      relu(x@W1+b1)@W2 + b2 (+residual), hT stays in SBUF
  - qk projections run in fp8 e4m3 DoubleRow (2 K-rows/cycle; weights
    pre-scaled x64 on host, descaled in the eviction).  fp8 elsewhere fails
    the 2e-2 gate (measured: ffn 2.7e-2, +v 2.0e-2); qk costs only +5e-4
    because softmax damps logit noise.
  - Residual spine is bf16 (error budget allows; halves DMA + SBUF).
  - Attention is batched 4 heads per step: scores for 4 SAME-PARITY heads
    (mixed 0/64 base-partitions into one PSUM bank crash the HW stack) land
    in ONE [128,512] PSUM bank; exp/mask/normalize run as [128,512]-wide
    ACT/DVE ops (amortizes the ~150-250ns fixed per-instruction engine cost).
    One STREAM_TRANSPOSE per 4 heads: wei is 32-block-diagonal (4 batches of
    T=32), so the DVE's per-32x32-block transpose IS the full transpose.
  - Software pipelining: iteration c interleaves, at PE-unit granularity, the
    dense front of chunk c (qk+v), attention of chunk c-1 (out-matmuls trail
    their scores by two 4-head batches so the softmax DVE chain never
    head-of-line blocks the PE queue), and the projection of chunk c-2.
  - v-bias folded into the proj bias on host (softmax rows sum to 1).
  - attn^T staged through per-parity DRAM tensors (the only way to move head
    outputs from partitions 0:64 into the 128-partition proj K-layout).
  - Pass-boundary overlap: pools shared across both layer passes (slots free
    as soon as the last user finishes, not at pass end), per-chunk DRAM
    tensors for the inter-pass activations (no whole-tensor dependencies),
    and a quarter of W_ff1 preloaded into SBUF slack during pass 2.
"""
import sys

if "/opt/trn_rl_repo" not in sys.path:
    sys.path.insert(0, "/opt/trn_rl_repo")

from contextlib import ExitStack

import numpy as np

import concourse.bacc as bacc
import concourse.mybir as mybir
import concourse.tile as tile

F32 = mybir.dt.float32
BF16 = mybir.dt.bfloat16
F8E4 = mybir.dt.float8e4
P = 128

QK_FP8 = True      # qk matmuls in e4m3 DoubleRow (2 K-rows/cycle)
W8_SCALE = 64.0    # weight pre-scale into e4m3 sweet spot; descaled at evict

E = 1024
H = 16
D = 64
T = 32
HD = H * D  # 1024
FF = 4 * E  # 4096
N_CORES = 8
B_FULL = 1024
C = 512  # token chunk (columns per dense matmul)

AF = mybir.ActivationFunctionType
ALU = mybir.AluOpType

# bisection flags (all-True = fastest path)
FLAGS = {
    "big_transpose": True,   # one [128,512] STREAM_TRANSPOSE vs 4x[128,128]
    "scores_onebank": True,  # 4 scores MMs into one PSUM bank vs 4 banks
    "do_attn": True,         # emit attention (else memset ostrip)
    "batched_evict": True,   # one [64,512] ACT evict vs 4x[64,128]
}

# Head processing order: 4-head batches of UNIFORM parity, so the 4 scores
# matmuls sharing one PSUM bank all use the same tile_position (mixed 0/64
# row-positions into one bank crash the HW stack).  Consecutive batches
# ALTERNATE parity so their MMs land on disjoint PE row halves and overlap.
HEAD_BATCHES = [(0, 2, 4, 6), (1, 3, 5, 7), (8, 10, 12, 14), (9, 11, 13, 15)]

# FFN split-K fp8: first *_K8 contraction rows run e4m3 DoubleRow, the rest
# bf16.  Both halves' weights are pre-scaled x64 on host so the partial sums
# share one PSUM scale; the eviction's scale=1/64 descales for free.
# Error budget (CPU sim): err^2 ~ 0.0073^2 + f1*0.0184^2 + f2*0.0211^2.
FFN1_K8 = 512   # of E=1024   (f1=0.5)
FFN2_K8 = 512   # of FF=4096  (f2=0.125)


def _rd(ap):
    """DRAM [R, t] viewed as [128, R//128, t] (row r = k*128 + p)."""
    return ap.rearrange("(k p) t -> p k t", p=P)


def _chunk(ap_or_list, c):
    """Chunk c of an activation: per-chunk DRAM tensor list or one tensor."""
    if isinstance(ap_or_list, list):
        return _rd(ap_or_list[c])
    return _rd(ap_or_list)[:, :, slice(c * C, (c + 1) * C)]


def _layer_pass(tc, psums, pools, mask_sb, xin_d, Wqk_d, Wv_d, Wp_d,
                bqk_d, bp_d, xout_d, tok,
                wqk_pre=None, prefetch_cb=None):
    """One attention layer: xout = xin + attn(xin) @ Wp + bp' (all bf16 I/O).

    Software-pipelined over chunks: iteration c emits, PE-interleaved at unit
    granularity, the dense front of chunk c (qk+v), the attention of chunk
    c-1, and the projection of chunk c-2.  Attention out-matmuls trail their
    scores by one 4-head batch so the softmax DVE chain never head-of-line
    blocks the PE queue.
    """
    nc = tc.nc
    mm_ps, s_ps_pool, o_ps_pool = psums
    n_chunks = tok // C
    n_grp = C // P  # groups of 4 batches per chunk
    scale = float(D) ** -0.5

    if True:
        wp = pools["wts"]
        if wqk_pre is not None:
            Wqk_sb = wqk_pre  # prefetched during the previous pass
        else:
            Wqk_sb = pools["wqk"].tile([P, E // P, 2 * HD],
                                       F8E4 if QK_FP8 else BF16, name="Wqk_sb")
        bqk_sb = wp.tile([P, 16], F32, name="bqk_sb")
        nc.sync.dma_start(bqk_sb[:], bqk_d[:])
        bp_sb = wp.tile([P, 8], F32, name="bp_sb")
        nc.sync.dma_start(bp_sb[:], bp_d[:])
        # Wv/Wp loads are deferred to just after the first x-chunk DMA so the
        # first qk matmuls aren't stuck behind them in the DMA queue.
        Wv_sb = wp.tile([P, HD // P, HD], BF16, name="Wv_sb")
        Wp_sb = wp.tile([P, HD // P, E], BF16, name="Wp_sb")

        x_pool = pools["x"]
        x8_pool = pools["x8"]
        qkT_pool = pools["qkT"]
        v_pool = pools["v"]
        w_pool = pools["w"]
        wT_pool = pools["wT"]
        st_pool = pools["st"]
        aT_pool = pools["aT"]
        xo_pool = pools["xo"]

        x_tiles, x8_tiles, qkT_tiles, v_tiles, aT_tiles = {}, {}, {}, {}, {}

        def qk_unit(c, m):
            ps = mm_ps.tile([P, C], F32, name="mm")
            if QK_FP8:
                x8 = x8_tiles[c]
                for k in range(4):
                    nc.tensor.matmul(
                        ps[:],
                        lhsT=Wqk_sb[:, 2 * k:2 * k + 2, m * P:(m + 1) * P],
                        rhs=x8[:, 2 * k:2 * k + 2, :],
                        perf_mode=mybir.MatmulPerfMode.DoubleRow,
                        start=(k == 0), stop=(k == 3),
                    )
                # evict on ACT (keeps DVE free for the softmax chains)
                nc.scalar.activation(
                    qkT_tiles[c][:, m, :], ps, AF.Identity,
                    bias=bqk_sb[:, m:m + 1], scale=1.0 / W8_SCALE)
            else:
                for k in range(8):
                    nc.tensor.matmul(
                        ps[:], lhsT=Wqk_sb[:, k, m * P:(m + 1) * P],
                        rhs=x_tiles[c][:, k, :], start=(k == 0), stop=(k == 7),
                    )
                nc.scalar.activation(
                    qkT_tiles[c][:, m, :], ps, AF.Identity,
                    bias=bqk_sb[:, m:m + 1])

        def v_unit(c, mt, n2):
            ps = mm_ps.tile([P, C], F32, name="mm")
            for k in range(8):
                nc.tensor.matmul(
                    ps[:], lhsT=x_tiles[c][:, k, mt * P:(mt + 1) * P],
                    rhs=Wv_sb[:, k, n2 * C:(n2 + 1) * C],
                    start=(k == 0), stop=(k == 7),
                )
            nc.scalar.copy(
                out=v_tiles[c][:, mt, n2 * C:(n2 + 1) * C], in_=ps)

        def proj_unit(c, m):
            ps = mm_ps.tile([P, C], F32, name="mm")
            for k in range(8):
                nc.tensor.matmul(
                    ps[:], lhsT=Wp_sb[:, k, m * P:(m + 1) * P],
                    rhs=aT_tiles[c][:, k, :], start=(k == 0), stop=(k == 7),
                )
            xo = xo_tiles[c]
            # fused (ps + bias) + residual in one DVE pass
            nc.vector.scalar_tensor_tensor(
                out=xo[:, m, :], in0=ps, scalar=bp_sb[:, m:m + 1],
                in1=x_tiles[c][:, m, :], op0=ALU.add, op1=ALU.add)

        xo_tiles = {}

        def scores_front(c, g, hb):
            """4 scores MMs + softmax chain; returns weiT tile."""
            qkT = qkT_tiles[c]
            gc = slice(g * P, (g + 1) * P)
            heads = HEAD_BATCHES[hb]
            s_ps = s_ps_pool.tile([P, 4 * P], F32, name="s_ps")
            for j, h in enumerate(heads):
                r0 = D * (h % 2)
                nc.tensor.matmul(
                    s_ps[:, j * P:(j + 1) * P],
                    lhsT=qkT[r0:r0 + D, h // 2, gc],
                    rhs=qkT[r0:r0 + D, 8 + h // 2, gc],
                    start=True, stop=True,
                )
            e_sb = w_pool.tile([P, 4 * P], BF16, name="e_sb")
            nc.scalar.activation(e_sb, s_ps, AF.Exp, scale=scale)
            wei = w_pool.tile([P, 4 * P], BF16, name="wei")
            # mask multiply on the (otherwise idle) GpSimd engine
            nc.gpsimd.tensor_tensor(out=wei, in0=e_sb, in1=mask_sb, op=ALU.mult)
            sums = st_pool.tile([P, 4], F32, name="sums")
            nc.vector.tensor_reduce(
                out=sums, in_=wei.rearrange("p (j t) -> p j t", j=4),
                op=ALU.add, axis=mybir.AxisListType.X,
            )
            rcp = st_pool.tile([P, 4], F32, name="rcp")
            nc.vector.reciprocal(rcp, sums)
            nc.vector.tensor_tensor(
                out=wei.rearrange("p (j t) -> p j t", j=4),
                in0=wei.rearrange("p (j t) -> p j t", j=4),
                in1=rcp.unsqueeze(-1).broadcast_to([P, 4, P]),
                op=ALU.mult,
            )
            weiT = wT_pool.tile([P, 4 * P], BF16, name="weiT")
            nc.vector.transpose(weiT, wei)  # 32-blockwise == full T here
            return weiT

        def out_back(c, g, hbe, weiT_e, weiT_o):
            """8 out MMs for a batch PAIR (even-parity hbe, odd hbe+1) into one
            bank: even heads -> psum partitions 0:64, odd heads -> 64:128 via
            col tiling.  Each head pair (2k, 2k+1) then IS aT feature block k,
            evicted straight to SBUF -- no DRAM stage roundtrip."""
            heads_e = HEAD_BATCHES[hbe]
            gc = slice(g * P, (g + 1) * P)
            o_ps = o_ps_pool.tile([P, 4 * P], F32, name="o_ps")
            for j, he in enumerate(heads_e):
                ho = he + 1
                nc.tensor.matmul(
                    o_ps[0:D, j * P:(j + 1) * P],
                    lhsT=v_tiles[c][:, g, he * D:(he + 1) * D],
                    rhs=weiT_e[:, j * P:(j + 1) * P],
                    start=True, stop=True,
                )
                nc.tensor.matmul(
                    o_ps[D:P, j * P:(j + 1) * P],
                    lhsT=v_tiles[c][:, g, ho * D:(ho + 1) * D],
                    rhs=weiT_o[:, j * P:(j + 1) * P],
                    start=True, stop=True, tile_position=(0, D),
                )
            if c not in aT_tiles:
                aT_tiles[c] = aT_pool.tile([P, HD // P, C], BF16, name="aT")
            k0 = heads_e[0] // 2  # feature block of the first pair
            nc.scalar.activation(
                aT_tiles[c][:, k0:k0 + 4, gc],
                o_ps[:].rearrange("p (j t) -> p j t", j=4),
                AF.Copy,
            )

        def load_x(c):
            if c in x_tiles or c >= n_chunks:
                return
            x_sb = x_pool.tile([P, E // P, C], BF16, name="x_sb")
            nc.sync.dma_start(x_sb[:], _chunk(xin_d, c))
            x_tiles[c] = x_sb
            if c == 0:
                if wqk_pre is None:
                    nc.sync.dma_start(Wqk_sb[:, :, 0:HD], _rd(Wqk_d)[:, :, 0:HD])
                    nc.sync.dma_start(Wqk_sb[:, :, HD:2 * HD], _rd(Wqk_d)[:, :, HD:2 * HD])
                nc.sync.dma_start(Wv_sb[:], _rd(Wv_d))
                nc.sync.dma_start(Wp_sb[:], _rd(Wp_d))

        def cast_x8(c):
            # deferred to the END of the previous iteration so these casts
            # never sit ahead of softmax work in the DVE queue while the
            # x-chunk DMA is still in flight
            if not QK_FP8 or c in x8_tiles or c >= n_chunks:
                return
            x8 = x8_pool.tile([P, E // P, C], F8E4, name="x8")
            for kq in range(4):
                nc.vector.tensor_copy(
                    out=x8[:, 2 * kq:2 * kq + 2, :],
                    in_=x_tiles[c][:, 2 * kq:2 * kq + 2, :])
            x8_tiles[c] = x8

        for c in range(n_chunks + 2):
            if c == 5 and prefetch_cb is not None:
                prefetch_cb()
            dense = []
            if c < n_chunks:
                load_x(c)
                load_x(c + 1)  # prefetch: next chunk's DMA a full iter early
                cast_x8(c)     # no-op except at c == 0
                qkT_tiles[c] = qkT_pool.tile(
                    [P, 16, C], F8E4 if QK_FP8 else BF16, name="qkT")
                v_tiles[c] = v_pool.tile([P, n_grp, HD], BF16, name="V")
                dense += [(qk_unit, (c, m)) for m in range(16)]
                dense += [(v_unit, (c, mt, n2))
                          for mt in range(n_grp) for n2 in range(HD // C)]
            if c >= 2:
                xo_tiles[c - 2] = xo_pool.tile([P, E // P, C], BF16, name="xo")
                dense += [(proj_unit, (c - 2, m)) for m in range(8)]

            if 1 <= c <= n_chunks:
                ca = c - 1
                pend = []
                weiT_even = None
                nd0, popped = len(dense), 0
                for ci, (g, hb) in enumerate(
                        (g, hb) for g in range(n_grp) for hb in range(4)):
                    weiT = scores_front(ca, g, hb)
                    # spread the dense units evenly across all 16 chains
                    want = (nd0 * (ci + 1) + 15) // 16 - popped
                    for _ in range(want):
                        if dense:
                            fn, args = dense.pop(0)
                            fn(*args)
                            popped += 1
                    if hb % 2:
                        if pend:
                            out_back(*pend.pop(0))
                        pend.append((ca, g, hb - 1, weiT_even, weiT))
                    else:
                        weiT_even = weiT
                for fn, args in dense:
                    fn(*args)
                for pd in pend:
                    out_back(*pd)
                qkT_tiles.pop(ca)
                v_tiles.pop(ca)
                x8_tiles.pop(ca, None)
            else:
                for fn, args in dense:
                    fn(*args)

            if c < n_chunks:
                cast_x8(c + 1)  # next chunk's fp8 cast, after this chunk's DVE
            if c >= 2:
                cp = c - 2
                # write on the gpsimd queue: keeps the sync queue (reads)
                # free of head-of-line blocking behind not-yet-ready writes
                nc.gpsimd.dma_start(_chunk(xout_d, cp), xo_tiles.pop(cp)[:])
                x_tiles.pop(cp)
                aT_tiles.pop(cp)


def _ffn_pass(tc, psums, pools, preload, xin_d, Wff1b_d, Wff28_d, Wff2b_d,
              out_d, tok):
    """out = xin + relu(xin @ W1 + b1) @ W2 + b2   (out is fp32).

    Split-K fp8: contraction rows < FFN*_K8 run e4m3 DoubleRow, the rest
    bf16.  ALL weights are pre-scaled x64 on host so both halves' partial
    sums share one PSUM; evictions apply scale=1/64.
    """
    nc = tc.nc
    mm_ps, _, _ = psums
    n_chunks = tok // C
    k81 = FFN1_K8 // P   # fp8 k-groups in ffn1 (x side)
    k82 = FFN2_K8 // P   # fp8 m-tiles of h / k-groups in ffn2

    with ExitStack() as ctx:
        W18_sb, b1_sb, b2_sb = preload
        wp = ctx.enter_context(tc.tile_pool(name="fwts", bufs=1))
        x_pool = pools["x"]
        x8_pool = ctx.enter_context(tc.tile_pool(name="fx8_pool", bufs=2))
        h8_pool = ctx.enter_context(tc.tile_pool(name="fh8_pool", bufs=1))
        hb_pool = ctx.enter_context(tc.tile_pool(name="fhb_pool", bufs=1))
        o_pool = ctx.enter_context(tc.tile_pool(name="fo_pool", bufs=3))

        # first x chunk before the remaining big weight loads
        x0_sb = x_pool.tile([P, E // P, C], BF16, name="x_sb")
        nc.sync.dma_start(x0_sb[:], _chunk(xin_d, 0))
        # column-chunked loads so the first m-tiles' matmuls dep only on
        # their own region, not the whole multi-MB transfer
        W1b_sb = wp.tile([P, E // P - k81, FF], BF16, name="W1b_sb")
        for q in range(4):
            cq = slice(q * FF // 4, (q + 1) * FF // 4)
            nc.sync.dma_start(W1b_sb[:, :, cq], _rd(Wff1b_d)[:, :, cq])
        # W2 is not needed until the first ffn2 (~30us in); load it behind W1
        W28_sb = wp.tile([P, k82, E], F8E4, name="W28_sb")
        nc.sync.dma_start(W28_sb[:], _rd(Wff28_d))
        W2b_sb = wp.tile([P, FF // P - k82, E], BF16, name="W2b_sb")
        for q in range(4):
            cq = slice(q * E // 4, (q + 1) * E // 4)
            nc.sync.dma_start(W2b_sb[:, :, cq], _rd(Wff2b_d)[:, :, cq])

        fx_tiles = {}
        fx8_tiles = {}

        def fload_x(c):
            if c in fx_tiles or c >= n_chunks:
                return
            if c == 0:
                t = x0_sb
            else:
                t = x_pool.tile([P, E // P, C], BF16, name="x_sb")
                nc.sync.dma_start(t[:], _chunk(xin_d, c))
            fx_tiles[c] = t

        def fcast_x8(c):
            if c in fx8_tiles or c >= n_chunks:
                return
            t = fx_tiles[c]
            x8 = x8_pool.tile([P, k81, C], F8E4, name="fx8")
            for kq in range(k81 // 2):
                nc.vector.tensor_copy(
                    out=x8[:, 2 * kq:2 * kq + 2, :],
                    in_=t[:, 2 * kq:2 * kq + 2, :])
            fx8_tiles[c] = x8

        for c in range(n_chunks):
            cols = slice(c * C, (c + 1) * C)
            fload_x(c)
            fload_x(c + 1)
            fcast_x8(c)  # no-op except at c == 0
            x_sb, x8_sb = fx_tiles[c], fx8_tiles[c]

            hT8 = h8_pool.tile([P, k82, C], F8E4, name="hT8")
            hTb = hb_pool.tile([P, FF // P - k82, C], BF16, name="hTb")
            for m in range(FF // P):
                ps = mm_ps.tile([P, C], F32, name="mm")
                for kq in range(k81 // 2):
                    nc.tensor.matmul(
                        ps[:],
                        lhsT=W18_sb[:, 2 * kq:2 * kq + 2, m * P:(m + 1) * P],
                        rhs=x8_sb[:, 2 * kq:2 * kq + 2, :],
                        perf_mode=mybir.MatmulPerfMode.DoubleRow,
                        start=(kq == 0), stop=False,
                    )
                for k in range(k81, E // P):
                    nc.tensor.matmul(
                        ps[:], lhsT=W1b_sb[:, k - k81, m * P:(m + 1) * P],
                        rhs=x_sb[:, k, :], start=False, stop=(k == E // P - 1),
                    )
                hs = hT8[:, m, :] if m < k82 else hTb[:, m - k82, :]
                nc.scalar.activation(
                    hs, ps, AF.Relu, bias=b1_sb[:, m:m + 1],
                    scale=1.0 / W8_SCALE)
                if m == FF // P - 1:
                    fcast_x8(c + 1)  # next chunk's fp8 cast, mid-chunk

            for m in range(E // P):
                ps = mm_ps.tile([P, C], F32, name="mm")
                for kq in range(k82 // 2):
                    nc.tensor.matmul(
                        ps[:],
                        lhsT=W28_sb[:, 2 * kq:2 * kq + 2, m * P:(m + 1) * P],
                        rhs=hT8[:, 2 * kq:2 * kq + 2, :],
                        perf_mode=mybir.MatmulPerfMode.DoubleRow,
                        start=(kq == 0), stop=False,
                    )
                for k in range(k82, FF // P):
                    nc.tensor.matmul(
                        ps[:], lhsT=W2b_sb[:, k - k82, m * P:(m + 1) * P],
                        rhs=hTb[:, k - k82, :],
                        start=False, stop=(k == FF // P - 1),
                    )
                ot = o_pool.tile([P, C], F32, name="ot")
                nc.vector.tensor_scalar(
                    ot, ps, 1.0 / W8_SCALE, b2_sb[:, m:m + 1],
                    ALU.mult, ALU.add)
                nc.vector.tensor_tensor(
                    out=ot, in0=ot, in1=x_sb[:, m, :], op=ALU.add)
                nc.gpsimd.dma_start(_rd(out_d)[:, m, cols], ot[:])


def build_kernel(b_shard):
    """Build the per-core Bass module for a batch shard of b_shard blocks."""
    tok = b_shard * T
    nc = bacc.Bacc(None, target_bir_lowering=False)
    with tile.TileContext(nc) as tc:
        with ExitStack() as ctx:
            dram = ctx.enter_context(tc.tile_pool(name="dram", bufs=1, space="DRAM"))

            def din(name, shape, dt=BF16):
                return dram.tile(shape, dt, kind="ExternalInput", uniquify=False, name=name)

            qk_dt = F8E4 if QK_FP8 else BF16
            xTb = din("xTb", [E, tok])
            Wqk1 = din("Wqk1", [E, 2 * HD], qk_dt); Wv1 = din("Wv1", [E, HD]); Wp1 = din("Wp1", [HD, E])
            Wqk2 = din("Wqk2", [E, 2 * HD], qk_dt); Wv2 = din("Wv2", [E, HD]); Wp2 = din("Wp2", [HD, E])
            Wff1_8 = din("Wff1_8", [FFN1_K8, FF], F8E4)
            Wff1_b = din("Wff1_b", [E - FFN1_K8, FF])
            Wff2_8 = din("Wff2_8", [FFN2_K8, E], F8E4)
            Wff2_b = din("Wff2_b", [FF - FFN2_K8, E])
            bqk1 = din("bqk1", [P, 16], F32); bp1 = din("bp1", [P, 8], F32)
            bqk2 = din("bqk2", [P, 16], F32); bp2 = din("bp2", [P, 8], F32)
            bff1 = din("bff1", [P, 32], F32); bff2 = din("bff2", [P, 8], F32)
            maskc = din("maskc", [P, 4 * P])

            outT = dram.tile([E, tok], F32, kind="ExternalOutput", uniquify=False, name="outT")

            n_ch = tok // C
            xT2b = [dram.tile([E, C], BF16, kind="Internal", uniquify=False, name=f"xT2b_{i}") for i in range(n_ch)]
            xT3b = [dram.tile([E, C], BF16, kind="Internal", uniquify=False, name=f"xT3b_{i}") for i in range(n_ch)]

            sb = 2 if FLAGS["scores_onebank"] else 1
            mm_ps = ctx.enter_context(tc.tile_pool(name="mm_ps", bufs=4, space="PSUM"))
            s_ps = ctx.enter_context(tc.tile_pool(name="s_ps", bufs=sb, space="PSUM"))
            o_ps = ctx.enter_context(tc.tile_pool(name="o_ps", bufs=sb, space="PSUM"))
            psums = (mm_ps, s_ps, o_ps)

            pools = {
                "x": ctx.enter_context(tc.tile_pool(name="x_pool", bufs=4)),
            }
            const_p = ctx.enter_context(tc.tile_pool(name="const", bufs=1))
            mask_sb = const_p.tile([P, 4 * P], BF16, name="mask_sb")
            nc.sync.dma_start(mask_sb[:], maskc[:])
            fpre = ctx.enter_context(tc.tile_pool(name="fwts_pre", bufs=1))

            with ExitStack() as lctx:
                for nm, bufs in (("wts", 1), ("wqk", 2), ("x8", 2), ("qkT", 2),
                                 ("v", 2), ("w", 3), ("wT", 4), ("st", 6),
                                 ("aT", 2), ("xo", 2)):
                    pools[nm] = lctx.enter_context(
                        tc.tile_pool(name=nm + "_pool", bufs=bufs))

                box = {}

                def pre_l2():
                    # prefetch layer-2 qk weights mid-layer-1 (2nd wqk slot)
                    t = pools["wqk"].tile([P, E // P, 2 * HD],
                                          F8E4 if QK_FP8 else BF16,
                                          name="Wqk_sb")
                    nc.sync.dma_start(t[:, :, 0:HD], _rd(Wqk2)[:, :, 0:HD])
                    nc.sync.dma_start(t[:, :, HD:2 * HD], _rd(Wqk2)[:, :, HD:2 * HD])
                    box["wqk2"] = t

                def pre_ffn():
                    # prefetch the ffn1 fp8 weights + biases mid-layer-2
                    # (W2's fp8 part loads at FFN start; not needed till ~25us in)
                    W18_sb = fpre.tile([P, FFN1_K8 // P, FF], F8E4, name="W18_sb")
                    nc.sync.dma_start(W18_sb[:], _rd(Wff1_8))
                    b1_sb = fpre.tile([P, FF // P], F32, name="b1_sb")
                    nc.sync.dma_start(b1_sb[:], bff1[:])
                    b2_sb = fpre.tile([P, E // P], F32, name="b2_sb")
                    nc.sync.dma_start(b2_sb[:], bff2[:])
                    box["ffn"] = (W18_sb, b1_sb, b2_sb)

                _layer_pass(tc, psums, pools, mask_sb, xTb, Wqk1, Wv1, Wp1,
                            bqk1, bp1, xT2b, tok, prefetch_cb=pre_l2)
                _layer_pass(tc, psums, pools, mask_sb, xT2b, Wqk2, Wv2, Wp2,
                            bqk2, bp2, xT3b, tok,
                            wqk_pre=box["wqk2"], prefetch_cb=pre_ffn)
            _ffn_pass(tc, psums, pools, box["ffn"], xT3b, Wff1_b, Wff2_8,
                      Wff2_b, outT, tok)

    nc.compile()
    return nc


# --------------------------------------------------------------------------
# Host-side wrapper
# --------------------------------------------------------------------------

import ml_dtypes

BF16_NP = ml_dtypes.bfloat16


def _w_heads(W):
    """[H, E, D] -> [E, H*D] contiguous bf16 (col = 64h + d)."""
    return np.ascontiguousarray(
        np.transpose(np.asarray(W), (1, 0, 2)).reshape(E, HD).astype(BF16_NP)
    )


def _b_tile(b, n_po):
    """[Dim] -> [128, n_po] bias tile (row r = po*128 + pi)."""
    b = np.asarray(b, dtype=np.float32).reshape(n_po, P)
    return np.ascontiguousarray(b.T)


def _causal_mask_tile():
    """0/1 bf16 mask [128, 512]: 4-head tile of block-diag causal [128,128]."""
    m = np.zeros((P, P), dtype=np.float32)
    for p in range(P):
        blk, t = p // T, p % T
        m[p, blk * T: blk * T + t + 1] = 1.0
    return np.ascontiguousarray(np.tile(m, (1, 4)).astype(BF16_NP))


def make_in_maps(inputs, b_shard=B_FULL // N_CORES, n_cores=N_CORES):
    F8_NP = ml_dtypes.float8_e4m3
    x = np.asarray(inputs["x"], dtype=np.float32)

    def _q8(w):
        return np.ascontiguousarray(
            np.clip(w * W8_SCALE, -240, 240).astype(F8_NP))

    W1 = np.asarray(inputs["W_ff1"], np.float32)
    W2 = np.asarray(inputs["W_ff2"], np.float32)
    shared = {
        "maskc": _causal_mask_tile(),
        # split-K fp8: both halves x64 so PSUM partial sums share one scale
        "Wff1_8": _q8(W1[:FFN1_K8]),
        "Wff1_b": np.ascontiguousarray((W1[FFN1_K8:] * W8_SCALE).astype(BF16_NP)),
        "Wff2_8": _q8(W2[:FFN2_K8]),
        "Wff2_b": np.ascontiguousarray((W2[FFN2_K8:] * W8_SCALE).astype(BF16_NP)),
        "bff1": _b_tile(inputs["b_ff1"], 32),
        "bff2": _b_tile(inputs["b_ff2"], 8),
    }
    for li in ("1", "2"):
        Wq = _w_heads(inputs["Wq" + li])
        Wk = _w_heads(inputs["Wk" + li])
        wqk = np.ascontiguousarray(np.concatenate([Wq, Wk], axis=1))
        if QK_FP8:
            wqk = np.clip(wqk.astype(np.float32) * W8_SCALE, -240, 240).astype(F8_NP)
        shared["Wqk" + li] = wqk
        shared["Wv" + li] = _w_heads(inputs["Wv" + li])
        Wp = np.asarray(inputs["Wp" + li], np.float32)
        shared["Wp" + li] = np.ascontiguousarray(Wp.astype(BF16_NP))
        bq = np.asarray(inputs["bq" + li], np.float32).reshape(HD)
        bk = np.asarray(inputs["bk" + li], np.float32).reshape(HD)
        shared["bqk" + li] = _b_tile(np.concatenate([bq, bk]), 16)
        # fold v-bias through the projection:  bp' = bp + bv @ Wp
        bv = np.asarray(inputs["bv" + li], np.float32).reshape(HD)
        bp = np.asarray(inputs["bp" + li], np.float32) + bv @ Wp
        shared["bp" + li] = _b_tile(bp, 8)

    in_maps = []
    for c in range(n_cores):
        xs = x[c * b_shard:(c + 1) * b_shard].reshape(b_shard * T, E)
        m = dict(shared)
        m["xTb"] = np.ascontiguousarray(xs.T.astype(BF16_NP))
        in_maps.append(m)
    return in_maps


_NC_CACHE = {}


def kernel(**inputs) -> np.ndarray:
    from concourse.bass_utils import run_bass_kernel_spmd

    b_shard = B_FULL // N_CORES
    if b_shard not in _NC_CACHE:
        _NC_CACHE[b_shard] = build_kernel(b_shard)
    nc = _NC_CACHE[b_shard]

    in_maps = make_in_maps(inputs)
    res = run_bass_kernel_spmd(nc, in_maps, core_ids=list(range(N_CORES)))

    out = np.empty((B_FULL, T, E), dtype=np.float32)
    for c in range(N_CORES):
        outT = res.results[c]["outT"]  # [E, tok]
        out[c * b_shard:(c + 1) * b_shard] = outT.T.reshape(b_shard, T, E)
    return out

